# revision 7
# baseline (speedup 1.0000x reference)
"""Trainium2 Bass kernel for the two-level softmax-pooled text/video retrieval head.

Computes, for text_feat [256,32,512], video_feat [256,16,512], text_mask [256,32]:
    out[a,b] = (t2v(a,b) + v2t(a,b)) / 2
where t2v/v2t are two-level softmax-weighted poolings of the cross token/frame
cosine similarity tensor logits[a,b,t,v] (see reference module).

Sharding: text axis A split across 8 NeuronCores (32 queries each); video
features replicated. Host does l2-normalization + transposition (layout prep);
the device does all einsum + softmax compute.

v8 design: on top of the v7 pipeline (see below), VALID-TOKEN COMPACTION.
Lengths are in [16,32) (mean ~23.4), so ~27% of the (q,t) rows are padding
that v7 computed and then masked away via the 0/1 selector values.  v8
instead packs only the valid tokens of each query into consecutive rows and
lets the (already data-dependent) selector matmuls carry the arbitrary
row->query map.  Queries are assigned to cores by greedy LPT balancing so
every core's row count fits the same SPMD program (749 rows max vs the 768
capacity of 3 m-pairs for the reference inputs; a 4-pair program is compiled
as fallback if some other input needs it).  This cuts the main loop from 32
to 24 (pair,n) tiles: einsum, exp/copy, XE, v-tree and selector work all
drop ~25% across PE/ACT/DVE.  Mask values are baked into the text features
(t_row *= mask) so non-binary masks would still be exact.

v7 pipeline (per core, N_PAIR=3: 768 (q,t) rows; B=256, V=16 -> 4096 (b,v)
cols; D=512):
  - inputs are bf16 (0.4% feature rounding, ~1.7% first-level softmax weight
    noise -- well inside the 2e-2 tolerance); halves input DMA.
  - m-tiles are processed in PAIRS: one [128,1024] PSUM tile (2 banks) per
    pair, so ACT exp / ACT X-copy / DVE XE-mult each run at [128,1024] width.
  - E and XE are bf16; the XE multiply runs in the DVE 2x_1p mode.
  - sum-over-v (t2v level 1) is a 4-step strided add-tree on DVE (bf16 2x).
  - sum-over-t (v2t level 1 den/num) stays on PE as selector matmuls (bf16,
    exact 0/1, 32-wide stationary -> region-scoped PSUM accumulation),
    software-pipelined behind the main matmuls.
  - loop order is pair-outer / n-tile-inner within each b-half, so the t2v
    second level for a pair runs as soon as its half's columns are done.
"""

import sys

import numpy as np

if "/opt/trn_rl_repo" not in sys.path:
    sys.path.insert(0, "/opt/trn_rl_repo")

A, T_TOK, B, V_FRM, D = 256, 32, 256, 16, 512
N_CORES = 8
A_LOC = A // N_CORES            # 32 queries per core
N_COLS = B * V_FRM              # 4096  (b,v) cols
N_KC = D // 128                 # 4 K-chunks
N_NT = N_COLS // 512            # 8 N-tiles (32 videos each)
TAU = 100.0
SHIFT = -30.0                   # global exp shift (softmax-invariant)
EPS = 1e-6

_PROGRAM_CACHE = {}


def _build_program(n_pairs=3, reps=1, variant=0):
    import contextlib

    import concourse.mybir as mybir
    import concourse.tile as tile
    from concourse import bacc

    N_PAIR = n_pairs
    N_MT = 2 * N_PAIR               # m-tiles of 128 rows
    M_ROWS = 128 * N_MT             # compacted (q,t) rows incl. padding

    f32 = mybir.dt.float32
    bf16 = mybir.dt.bfloat16
    EXP = mybir.ActivationFunctionType.Exp
    CPY = mybir.ActivationFunctionType.Copy
    MUL = mybir.AluOpType.mult
    ADD = mybir.AluOpType.add
    AX = mybir.AxisListType.X

    nc = bacc.Bacc("TRN2", target_bir_lowering=False, debug=False)

    tT_d = nc.dram_tensor("tT", [D, M_ROWS], bf16, kind="ExternalInput")
    vT_d = nc.dram_tensor("vT", [D, N_COLS], bf16, kind="ExternalInput")
    sel_d = nc.dram_tensor("sel", [128, N_MT * 32], bf16, kind="ExternalInput")
    # bias width varies with reps/variant so each build gets a distinct HLO
    # hash (the NEFF cache otherwise silently reuses the first-compiled
    # program)
    bias_cols = 1 + (reps - 1) + 7 * variant
    bias_d = nc.dram_tensor("bias", [128, bias_cols], f32, kind="ExternalInput")
    out_d = nc.dram_tensor("out", [A_LOC, B], f32, kind="ExternalOutput")

    with tile.TileContext(nc) as tc, contextlib.ExitStack() as ctx:
        persist = ctx.enter_context(tc.tile_pool(name="persist", bufs=1))
        ps_pool = ctx.enter_context(tc.tile_pool(name="ps", bufs=2, space="PSUM"))
        dn2_pool = ctx.enter_context(tc.tile_pool(name="dn2", bufs=1, space="PSUM"))
        dn3_pool = ctx.enter_context(tc.tile_pool(name="dn3", bufs=1, space="PSUM"))
        e_pool = ctx.enter_context(tc.tile_pool(name="e", bufs=10))
        tr_pool = ctx.enter_context(tc.tile_pool(name="tr", bufs=3))
        xb_pool = ctx.enter_context(tc.tile_pool(name="xb", bufs=3))
        t2v_pool = ctx.enter_context(tc.tile_pool(name="t2v", bufs=2))
        w_pool = ctx.enter_context(tc.tile_pool(name="w", bufs=3))
        v_pool = ctx.enter_context(tc.tile_pool(name="v2", bufs=2))

        # ---- persistent inputs. Four issue queues in parallel (SP/DVE/ACT
        # HWDGE + Pool SWDGE) so descriptor generation (~0.6-1us per DMA) is
        # not the startup bottleneck.  Critical-path tiles (first pair's text
        # chunks, the n=0/1 video tiles) go out as small single transfers in
        # consumption order; everything later is batched into wide transfers.
        tt_tiles = []
        vt_tiles = {}
        for k in range(N_KC):
            t_ = persist.tile([128, M_ROWS], bf16, tag=f"tt_{k}")
            tt_tiles.append(t_)

        def vt_chunk(k, n0, n1, q):
            t_ = persist.tile([128, 512 * (n1 - n0)], bf16, tag=f"vt_{k}_{n0}")
            q.dma_start(out=t_[:],
                        in_=vT_d.ap()[128 * k:128 * (k + 1),
                                      512 * n0:512 * n1])
            for n in range(n0, n1):
                vt_tiles[(k, n)] = t_[:, 512 * (n - n0):512 * (n - n0 + 1)]

        # text heads (first pair) + first video tiles: smallest possible
        # critical path to the first matmul.  HWDGE queues: SP carries the
        # text, ACT carries the first-half video; the Pool SWDGE queue takes
        # the constants + late n-tiles.
        tt_head = min(256, M_ROWS)
        for k in range(N_KC):
            nc.sync.dma_start(out=tt_tiles[k][:, 0:tt_head],
                              in_=tT_d.ap()[128 * k:128 * (k + 1), 0:tt_head])
            vt_chunk(k, 0, 1, nc.scalar)
        vt_chunk(2, 1, 2, nc.gpsimd)
        vt_chunk(3, 1, 2, nc.gpsimd)
        bias_sb = persist.tile([128, bias_cols], f32, tag="bias")
        nc.gpsimd.dma_start(out=bias_sb[:], in_=bias_d.ap())
        sel_sb = persist.tile([128, N_MT * 32], bf16, tag="sel")
        nc.gpsimd.dma_start(out=sel_sb[:], in_=sel_d.ap())
        vt_chunk(0, 1, 2, nc.scalar)
        vt_chunk(1, 1, 2, nc.scalar)
        # warm the ACT spline tables (exp) while the first einsum runs: the
        # auto-inserted table load lands before this dependency-free dummy
        warm = persist.tile([128, 1], f32, tag="warm")
        nc.scalar.activation(warm[:], nc.const_aps.tensor(0.0, (128, 1)),
                             EXP, bias=0.0, scale=1.0)
        if M_ROWS > tt_head:
            for k in range(N_KC):
                nc.sync.dma_start(
                    out=tt_tiles[k][:, tt_head:M_ROWS],
                    in_=tT_d.ap()[128 * k:128 * (k + 1), tt_head:M_ROWS])
        vt_chunk(0, 2, 4, nc.scalar)
        vt_chunk(1, 2, 4, nc.scalar)
        vt_chunk(2, 2, 4, nc.gpsimd)
        vt_chunk(3, 2, 4, nc.gpsimd)
        for k in range(N_KC):
            vt_chunk(k, 4, N_NT, nc.gpsimd)

        # S|N accumulator, side-major: col = side*(256*N_MT) + m*256 + n*32 + b
        sn_all = persist.tile([128, 2 * N_MT * 256], f32, tag="sn_all")
        # final staging, (j,q)-partition layout: row = 32j+q, col = 32h+b'
        # (video index = 128h + 32j + b'); avoids any partition-crossing DMA
        # until the single strided output DMA
        vt2_full = persist.tile([128, 64], f32, tag="vt2_full")
        bias0 = bias_sb[:, 0:1]

        def issue_sel(den2, num2, item):
            """The 4 over-t selector matmuls for one queued pair-tile.
            32-wide stationary: band j of den2/num2 is its own region-scoped
            accumulation group, complete at (mp==N_PAIR-1, mi==1)."""
            j, mp, exe = item
            for mi in range(2):
                m = 2 * mp + mi
                selm = sel_sb[:, 32 * m:32 * (m + 1)]
                band = slice(32 * j, 32 * (j + 1))
                nc.tensor.matmul(
                    den2[band, :], selm, exe[:, 512 * mi:512 * (mi + 1)],
                    start=(mp == 0 and mi == 0), stop=(mp == N_PAIR - 1 and mi == 1),
                    skip_group_check=True, tile_position=(0, 32 * j),
                )
                nc.tensor.matmul(
                    num2[band, :], selm, exe[:, 1024 + 512 * mi:1536 + 512 * mi],
                    start=(mp == 0 and mi == 0), stop=(mp == N_PAIR - 1 and mi == 1),
                    skip_group_check=True, tile_position=(0, 32 * j),
                )

        for _rep in range(reps):
            # t2v second-level accumulators in the (j,q)/(h,b') layout:
            # row = 32j+q, col = 32h+b'
            den3 = dn3_pool.tile([128, 64], f32, tag="den3")
            num3 = dn3_pool.tile([128, 64], f32, tag="num3")

            # --- phase-2 (t2v second level) for one m-tile and one b-half;
            # interleaved into the main loop right where its inputs are
            # ready.  Split into an a-stage (DVE ratio + ACT exp) and a
            # b-stage (DVE weight product + PE matmuls) so tail emission can
            # interleave several chains without idling DVE on ACT hops.
            def t2v_level2_a(m, h):
                mb, hb = m * 256, 128 * h
                s1 = sn_all[:, mb + hb:mb + hb + 128]
                n1 = sn_all[:, 256 * N_MT + mb + hb:256 * N_MT + mb + hb + 128]
                rs1 = t2v_pool.tile([128, 128], f32, tag="rs1")
                nc.vector.reciprocal(rs1[:], s1)
                t2v_t = t2v_pool.tile([128, 128], f32, tag="t2v_t")
                nc.vector.tensor_tensor(t2v_t[:], n1, rs1[:], op=MUL)
                w_t = w_pool.tile([128, 128], bf16, tag="w_t")
                nc.scalar.activation(w_t[:], t2v_t[:], EXP, bias=bias0,
                                     scale=TAU)
                return t2v_t, w_t

            def t2v_level2_b(m, h, t2v_t, w_t):
                xw_t = w_pool.tile([128, 128], bf16, tag="xw_t")
                with nc.allow_low_precision(reason="bf16 softmax weights"):
                    nc.vector.tensor_tensor(xw_t[:], t2v_t[:], w_t[:], op=MUL)
                selm = sel_sb[:, 32 * m:32 * (m + 1)]
                for j in range(4):
                    band = slice(32 * j, 32 * (j + 1))
                    hcol = slice(32 * h, 32 * h + 32)
                    wcol = slice(32 * j, 32 * (j + 1))
                    nc.tensor.matmul(
                        den3[band, hcol], selm, w_t[:, wcol],
                        start=(m == 0), stop=(m == N_MT - 1),
                        skip_group_check=True, tile_position=(0, 32 * j),
                    )
                    nc.tensor.matmul(
                        num3[band, hcol], selm, xw_t[:, wcol],
                        start=(m == 0), stop=(m == N_MT - 1),
                        skip_group_check=True, tile_position=(0, 32 * j),
                    )

            def t2v_level2(m, h):
                t2v_t, w_t = t2v_level2_a(m, h)
                t2v_level2_b(m, h, t2v_t, w_t)

            # ---- main loop: halves (b 0:128 / 128:256) x M-pairs x N-tiles
            for h in range(2):
                den2 = dn2_pool.tile([128, 512], f32, tag="den2")
                num2 = dn2_pool.tile([128, 512], f32, tag="num2")
                sel_queue = []   # software pipeline: sel MMs run 2 tiles late
                for mp in range(N_PAIR):
                    for j in range(4):
                        n = 4 * h + j
                        ps = ps_pool.tile([128, 1024], f32, tag="ps")
                        for mi in range(2):
                            m = 2 * mp + mi
                            for k in range(N_KC):
                                nc.tensor.matmul(
                                    ps[:, 512 * mi:512 * (mi + 1)],
                                    tt_tiles[k][:, 128 * m:128 * (m + 1)],
                                    vt_tiles[(k, n)],
                                    start=(k == 0),
                                    stop=(k == N_KC - 1),
                                )
                        # E|XE for the pair: [E(m0) E(m1) XE(m0) XE(m1)]
                        exe = e_pool.tile([128, 2048], bf16, tag="exe")
                        # the last pair of the half gates the v2t second
                        # level: split exp/mult per m-tile and multiply
                        # straight from PSUM (1x mode) so its selector
                        # matmuls issue ~1us earlier
                        tail_pair = (mp == N_PAIR - 1 and j == 3)
                        with nc.allow_low_precision(reason="bf16 E/XE"):
                            if tail_pair:
                                for mi in range(2):
                                    ms = slice(512 * mi, 512 * mi + 512)
                                    xs = slice(1024 + 512 * mi,
                                               1536 + 512 * mi)
                                    nc.scalar.activation(
                                        exe[:, ms], ps[:, ms], EXP,
                                        bias=bias0, scale=TAU)
                                    nc.vector.tensor_tensor(
                                        exe[:, xs], ps[:, ms], exe[:, ms],
                                        op=MUL)
                            else:
                                nc.scalar.activation(
                                    exe[:, 0:1024], ps[:], EXP, bias=bias0,
                                    scale=TAU)
                                xbt = xb_pool.tile([128, 1024], bf16,
                                                   tag="xbt")
                                nc.scalar.activation(xbt[:], ps[:], CPY)
                                nc.vector.tensor_tensor(
                                    exe[:, 1024:2048], xbt[:], exe[:, 0:1024],
                                    op=MUL)
                        # queue the 4 selector matmuls (over-t sums on PE),
                        # issued a few tiles later so their ACT/DVE deps are
                        # done by the time PE reaches them.  In the last pair
                        # the queue drains 2/tile so den2/num2 close right
                        # after the final XE instead of 4 items later.
                        sel_queue.append((j, mp, exe))
                        if mp == N_PAIR - 1:
                            for _ in range(2):
                                if sel_queue:
                                    issue_sel(den2, num2, sel_queue.pop(0))
                        elif len(sel_queue) > 4:
                            issue_sel(den2, num2, sel_queue.pop(0))
                        # sum over v: 4-step strided bf16 add-tree on DVE.
                        # tr: [0:1024]=(c4,b32,v8)  [1024:1536]=(c,b,4)
                        #     [1536:1792]=(c,b,2)
                        tr = tr_pool.tile([128, 1792], bf16, tag="tr")
                        ev16 = exe[:].rearrange("p (c b v) -> p c b v", c=4,
                                                v=16)
                        t8 = tr[:, 0:1024].rearrange("p (c b v) -> p c b v",
                                                     c=4, v=8)
                        t4 = tr[:, 1024:1536].rearrange("p (c b v) -> p c b v",
                                                        c=4, v=4)
                        t2 = tr[:, 1536:1792].rearrange("p (c b v) -> p c b v",
                                                        c=4, v=2)
                        with nc.allow_low_precision(reason="bf16 v-tree"):
                            nc.vector.tensor_tensor(
                                t8[:], ev16[:, :, :, 0:8], ev16[:, :, :, 8:16],
                                op=ADD)
                            nc.vector.tensor_tensor(
                                t4[:], t8[:, :, :, 0:4], t8[:, :, :, 4:8],
                                op=ADD)
                            nc.vector.tensor_tensor(
                                t2[:], t4[:, :, :, 0:2], t4[:, :, :, 2:4],
                                op=ADD)
                        # final add writes fp32 S|N: col = s*(256*N_MT)+m*256+n*32+b
                        snv = sn_all[:].rearrange(
                            "p (s m nb) -> p s m nb", s=2, m=N_MT)[
                            :, :, 2 * mp:2 * mp + 2, 32 * n:32 * (n + 1)]
                        nc.vector.tensor_tensor(
                            snv, t2[:, :, :, 0:1], t2[:, :, :, 1:2], op=ADD)
                    # this pair's half-columns are complete: run its t2v
                    # second level overlapped with the remaining pairs (the
                    # last pair's runs after the v2t chain below, which is
                    # the longer critical path at the end of the half)
                    if mp < N_PAIR - 1:
                        t2v_level2(2 * mp, h)
                        t2v_level2(2 * mp + 1, h)
                while sel_queue:
                    issue_sel(den2, num2, sel_queue.pop(0))

                # ---- second level of v2t for this half (softmax over v),
                # split into two column groups (b' 0:16 / 16:32) so the serial
                # chain pipelines and the last-half tail is ~2x shorter ----
                rden2 = v_pool.tile([128, 512], f32, tag="rden2")
                v_t = v_pool.tile([128, 512], f32, tag="v_t")
                exev = v_pool.tile([128, 1024], bf16, tag="exev")
                snv_t = v_pool.tile([128, 64], f32, tag="snv_t")
                rsv_t = v_pool.tile([128, 32], f32, tag="rsv_t")
                ev16 = exev[:].rearrange("p (c b v) -> p c b v", c=2, v=16)
                sn8 = snv_t[:].rearrange("p (c b) -> p c b", c=2)

                def v2t_a(cg):
                    cs = slice(256 * cg, 256 * cg + 256)
                    nc.vector.reciprocal(rden2[:, cs], den2[:, cs])
                    nc.vector.tensor_tensor(
                        v_t[:, cs], num2[:, cs], rden2[:, cs], op=MUL)
                    nc.scalar.activation(
                        exev[:, cs], v_t[:, cs], EXP, bias=bias0, scale=TAU)

                def v2t_b(cg):
                    cs = slice(256 * cg, 256 * cg + 256)
                    bs = slice(16 * cg, 16 * cg + 16)
                    with nc.allow_low_precision(reason="bf16 Ev/XEv"):
                        nc.vector.tensor_tensor(
                            exev[:, 512 + 256 * cg:768 + 256 * cg],
                            v_t[:, cs], exev[:, cs], op=MUL)
                    # single reduce (no fast modes either way) instead of an
                    # add-tree: 1 instruction of latency on the tail chain
                    nc.vector.reduce_sum(
                        out=sn8[:, :, bs],
                        in_=ev16[:, :, bs, :],
                        axis=AX)
                    nc.vector.reciprocal(rsv_t[:, bs], snv_t[:, bs])
                    # vt2 = 0.5 * Nv / Sv  (the final /2 folded in here),
                    # written straight into the (j,q)/(h,b') staging tile
                    nc.vector.scalar_tensor_tensor(
                        out=vt2_full[:, 32 * h + 16 * cg:32 * h + 16 * cg + 16],
                        in0=snv_t[:, 32:64][:, bs], scalar=0.5,
                        in1=rsv_t[:, bs], op0=MUL, op1=MUL,
                    )

                # interleave the v2t chains, the deferred last pair's t2v
                # level 2, and phase 3 so DVE always has ready work while
                # ACT exps round-trip
                m_a, m_b = 2 * (N_PAIR - 1), 2 * N_PAIR - 1
                v2t_a(0)
                ta = t2v_level2_a(m_a, h)
                tb = t2v_level2_a(m_b, h)
                v2t_a(1)
                v2t_b(0)
                t2v_level2_b(m_a, h, *ta)
                t2v_level2_b(m_b, h, *tb)
                v2t_b(1)

                # ---- phase 3 for this half: t2v2 = 0.5*Num3/Den3, combine
                # with v2t, and DMA out[q, 128h+32j+b'].  den3/num3's h-region
                # groups stop at m==N_MT-1 just above, so the h=0 finale fully
                # overlaps the h=1 main loop. ----
                hcol = slice(32 * h, 32 * h + 32)
                rden3 = t2v_pool.tile([128, 32], f32, tag="rden3")
                nc.vector.reciprocal(rden3[:], den3[:, hcol])
                t2v2 = t2v_pool.tile([128, 32], f32, tag="t2v2")
                nc.vector.scalar_tensor_tensor(
                    out=t2v2[:], in0=num3[:, hcol], scalar=0.5, in1=rden3[:],
                    op0=MUL, op1=MUL,
                )
                out_sb = t2v_pool.tile([128, 32], f32, tag="out_sb")
                nc.vector.tensor_tensor(out_sb[:], t2v2[:],
                                        vt2_full[:, hcol], op=ADD)
                out_ap = out_d.ap().rearrange(
                    "q (h j b) -> h j q b", h=2, j=4)[h:h + 1]
                nc.sync.dma_start(out=out_ap, in_=out_sb[:])

    nc.compile()
    return nc


def _get_program(n_pairs=3, reps=1, variant=0, **_ignored):
    key = (n_pairs, reps, variant)
    if key not in _PROGRAM_CACHE:
        _PROGRAM_CACHE[key] = _build_program(n_pairs, reps, variant)
    return _PROGRAM_CACHE[key]


def _l2norm(a):
    n = np.linalg.norm(a, axis=-1, keepdims=True)
    return a / np.maximum(n, EPS)


def _bf16(a):
    import ml_dtypes
    return a.astype(ml_dtypes.bfloat16)


def _assign_queries(lengths):
    """Greedy LPT: partition the A queries into N_CORES groups of A_LOC,
    minimizing the max total token count per core."""
    order = np.argsort(-lengths, kind="stable")
    core_sum = np.zeros(N_CORES, np.int64)
    core_cnt = np.zeros(N_CORES, np.int64)
    groups = [[] for _ in range(N_CORES)]
    for q in order:
        open_c = np.nonzero(core_cnt < A_LOC)[0]
        c = open_c[np.argmin(core_sum[open_c])]
        groups[c].append(int(q))
        core_sum[c] += lengths[q]
        core_cnt[c] += 1
    return groups, int(core_sum.max())


def prepare_inputs(text_feat, video_feat, text_mask):
    """Host-side shard/layout prep with valid-token compaction.
    Returns (in_maps for the 8 cores, query permutation, n_pairs)."""
    t = _l2norm(text_feat.astype(np.float32))          # [A, T, D]
    v = _l2norm(video_feat.astype(np.float32))         # [B, V, D]
    mask = text_mask.astype(np.float32)

    # bake mask values into the text rows: logits = (mask*t) . v exactly
    tm = t * mask[:, :, None]

    lengths = (mask != 0.0).sum(axis=1).astype(np.int64)   # valid tokens per query
    groups, max_rows = _assign_queries(lengths)
    n_pairs = max(1, -(-max_rows // 256))                   # ceil to 256-row pairs
    m_rows = 256 * n_pairs
    n_mt = 2 * n_pairs

    # video: [B, V, D] -> [D, B*V], shared by all cores
    vT = _bf16(np.ascontiguousarray(v.reshape(B * V_FRM, D).T))

    in_maps = []
    perm = np.empty(A, np.int64)
    for c in range(N_CORES):
        qs = groups[c]
        perm[c * A_LOC:(c + 1) * A_LOC] = qs
        rows = np.zeros((m_rows, D), np.float32)
        sel = np.zeros((128, n_mt * 32), np.float32)
        r = 0
        for jloc, q in enumerate(qs):
            idx = np.nonzero(mask[q] != 0.0)[0]
            k = len(idx)
            rows[r:r + k] = tm[q, idx]
            rr = np.arange(r, r + k)
            sel[rr % 128, (rr // 128) * 32 + jloc] = 1.0
            r += k
        tT = _bf16(np.ascontiguousarray(rows.T))            # [D, m_rows]
        bias = np.full((128, 1), SHIFT, np.float32)
        in_maps.append({"tT": tT, "vT": vT, "sel": _bf16(sel), "bias": bias})
    return in_maps, perm, n_pairs


def run(in_maps, n_pairs=3, trace=False, reps=1, variant=0, **kwargs):
    import concourse.mybir as mybir
    from concourse import bass_utils

    nc = _get_program(n_pairs=n_pairs, reps=reps, variant=variant)
    # pad inputs to the program's declared shapes (bias width varies by build)
    shapes = {}
    for alloc in nc.m.functions[0].allocations:
        if isinstance(alloc, mybir.MemoryLocationSet) and alloc.kind == "ExternalInput":
            shapes[alloc.memorylocations[0].name] = tuple(alloc.tensor_shape)
    fixed = []
    for m in in_maps:
        mm = {}
        for k, v in m.items():
            shp = shapes.get(k, tuple(v.shape))
            if tuple(v.shape) != shp:
                out = np.full(shp, SHIFT if k == "bias" else 0.0, v.dtype)
                sl = tuple(slice(0, min(s, t)) for s, t in zip(v.shape, shp))
                out[sl] = v[sl]
                mm[k] = out
            else:
                mm[k] = v
        fixed.append(mm)
    return bass_utils.run_bass_kernel_spmd(
        nc, fixed, core_ids=list(range(N_CORES)), trace=trace, **kwargs
    )


def kernel(text_feat, video_feat, text_mask):
    in_maps, perm, n_pairs = prepare_inputs(
        np.asarray(text_feat), np.asarray(video_feat), np.asarray(text_mask)
    )
    res = run(in_maps, n_pairs=n_pairs)
    packed = np.concatenate([res.results[c]["out"] for c in range(N_CORES)], axis=0)
    out = np.empty_like(packed)
    out[perm] = packed                      # un-permute queries to original order
    return out.astype(np.float32)


# revision 31
# speedup vs baseline: 1.0806x; 1.0806x over previous
"""Trainium2 Bass kernel for the two-level softmax-pooled text/video retrieval head.

Computes, for text_feat [256,32,512], video_feat [256,16,512], text_mask [256,32]:
    out[a,b] = (t2v(a,b) + v2t(a,b)) / 2
where t2v/v2t are two-level softmax-weighted poolings of the cross token/frame
cosine similarity tensor logits[a,b,t,v] (see reference module).

Sharding: text axis A split across 8 NeuronCores (32 queries each); video
features replicated. Host does l2-normalization + transposition (layout prep);
the device does all einsum + softmax compute.

v8 design: on top of the v7 pipeline (see below), VALID-TOKEN COMPACTION.
Lengths are in [16,32) (mean ~23.4), so ~27% of the (q,t) rows are padding
that v7 computed and then masked away via the 0/1 selector values.  v8
instead packs only the valid tokens of each query into consecutive rows and
lets the (already data-dependent) selector matmuls carry the arbitrary
row->query map.  Queries are assigned to cores by greedy LPT balancing so
every core's row count fits the same SPMD program (749 rows max vs the 768
capacity of 3 m-pairs for the reference inputs; a 4-pair program is compiled
as fallback if some other input needs it).  This cuts the main loop from 32
to 24 (pair,n) tiles: einsum, exp/copy, XE, v-tree and selector work all
drop ~25% across PE/ACT/DVE.  Mask values are baked into the text features
(t_row *= mask) so non-binary masks would still be exact.

v7 pipeline (per core, N_PAIR=3: 768 (q,t) rows; B=256, V=16 -> 4096 (b,v)
cols; D=512):
  - inputs are bf16 (0.4% feature rounding, ~1.7% first-level softmax weight
    noise -- well inside the 2e-2 tolerance); halves input DMA.
  - m-tiles are processed in PAIRS: one [128,1024] PSUM tile (2 banks) per
    pair, so ACT exp / ACT X-copy / DVE XE-mult each run at [128,1024] width.
  - E and XE are bf16; the XE multiply runs in the DVE 2x_1p mode.
  - sum-over-v (t2v level 1) is a 4-step strided add-tree on DVE (bf16 2x).
  - sum-over-t (v2t level 1 den/num) stays on PE as selector matmuls (bf16,
    exact 0/1, 32-wide stationary -> region-scoped PSUM accumulation),
    software-pipelined behind the main matmuls.
  - loop order is pair-outer / n-tile-inner within each b-half, so the t2v
    second level for a pair runs as soon as its half's columns are done.
"""

import sys

import numpy as np

if "/opt/trn_rl_repo" not in sys.path:
    sys.path.insert(0, "/opt/trn_rl_repo")

A, T_TOK, B, V_FRM, D = 256, 32, 256, 16, 512
N_CORES = 8
A_LOC = A // N_CORES            # 32 queries per core
N_COLS = B * V_FRM              # 4096  (b,v) cols
N_KC = D // 128                 # 4 K-chunks
N_NT = N_COLS // 512            # 8 N-tiles (32 videos each)
TAU = 100.0
SHIFT = -30.0                   # global exp shift (softmax-invariant)
EPS = 1e-6

_PROGRAM_CACHE = {}

# build-time scheduling knobs (A/B tuning; defaults = shipped config)
_TUNE = {"flush2": False, "dma_new": False, "pre_pop": False, "tail_order": 0,
         "e_bufs": 13, "skip": ""}


def _build_program(n_pairs=3, reps=1, variant=0):
    import contextlib

    import concourse.mybir as mybir
    import concourse.tile as tile
    from concourse import bacc

    N_PAIR = n_pairs
    N_MT = 2 * N_PAIR               # m-tiles of 128 rows
    M_ROWS = 128 * N_MT             # compacted (q,t) rows incl. padding

    f32 = mybir.dt.float32
    bf16 = mybir.dt.bfloat16
    EXP = mybir.ActivationFunctionType.Exp
    CPY = mybir.ActivationFunctionType.Copy
    MUL = mybir.AluOpType.mult
    ADD = mybir.AluOpType.add
    DIV = mybir.AluOpType.divide
    AX = mybir.AxisListType.X
    # The final /2 is folded into XE at creation (X-copy scale 0.5), so every
    # numerator downstream is pre-halved and each num/den ratio site becomes a
    # single DVE divide; the softmax exps compensate with scale 2*TAU.
    TAU2 = 2.0 * TAU

    nc = bacc.Bacc("TRN2", target_bir_lowering=False, debug=False)

    tT_d = nc.dram_tensor("tT", [D, M_ROWS], bf16, kind="ExternalInput")
    vT_d = nc.dram_tensor("vT", [D, N_COLS], bf16, kind="ExternalInput")
    sel_d = nc.dram_tensor("sel", [128, N_MT * 32], bf16, kind="ExternalInput")
    # bias width varies with reps/variant so each build gets a distinct HLO
    # hash (the NEFF cache otherwise silently reuses the first-compiled
    # program)
    bias_cols = 1 + (reps - 1) + 7 * variant
    bias_d = nc.dram_tensor("bias", [128, bias_cols], f32, kind="ExternalInput")
    out_d = nc.dram_tensor("out", [A_LOC, B], f32, kind="ExternalOutput")

    with tile.TileContext(nc) as tc, contextlib.ExitStack() as ctx:
        persist = ctx.enter_context(tc.tile_pool(name="persist", bufs=1))
        ps_pool = ctx.enter_context(tc.tile_pool(name="ps", bufs=2, space="PSUM"))
        dn2_pool = ctx.enter_context(tc.tile_pool(name="dn2", bufs=1, space="PSUM"))
        dn3_pool = ctx.enter_context(tc.tile_pool(name="dn3", bufs=1, space="PSUM"))
        e_pool = ctx.enter_context(tc.tile_pool(name="e", bufs=_TUNE["e_bufs"]))
        tr_pool = ctx.enter_context(tc.tile_pool(name="tr", bufs=3))
        xb_pool = ctx.enter_context(tc.tile_pool(name="xb", bufs=3))
        t2v_pool = ctx.enter_context(tc.tile_pool(name="t2v", bufs=2))
        w_pool = ctx.enter_context(tc.tile_pool(name="w", bufs=3))
        v_pool = ctx.enter_context(tc.tile_pool(name="v2", bufs=2))

        # ---- persistent inputs. Four issue queues in parallel (SP/DVE/ACT
        # HWDGE + Pool SWDGE) so descriptor generation (~0.6-1us per DMA) is
        # not the startup bottleneck.  Critical-path tiles (first pair's text
        # chunks, the n=0/1 video tiles) go out as small single transfers in
        # consumption order; everything later is batched into wide transfers.
        tt_tiles = []
        vt_tiles = {}
        for k in range(N_KC):
            t_ = persist.tile([128, M_ROWS], bf16, tag=f"tt_{k}")
            tt_tiles.append(t_)

        def vt_chunk(k, n0, n1, q):
            t_ = persist.tile([128, 512 * (n1 - n0)], bf16, tag=f"vt_{k}_{n0}")
            q.dma_start(out=t_[:],
                        in_=vT_d.ap()[128 * k:128 * (k + 1),
                                      512 * n0:512 * n1])
            for n in range(n0, n1):
                vt_tiles[(k, n)] = t_[:, 512 * (n - n0):512 * (n - n0 + 1)]

        if _TUNE["dma_new"]:
            # text heads (first pair) + first video tiles: smallest possible
            # critical path to the first matmul.  HWDGE queues: SP carries the
            # text, ACT carries the first-half video; the Pool SWDGE queue
            # takes the constants + late n-tiles.
            tt_head = min(256, M_ROWS)
            for k in range(N_KC):
                nc.sync.dma_start(out=tt_tiles[k][:, 0:tt_head],
                                  in_=tT_d.ap()[128 * k:128 * (k + 1),
                                                0:tt_head])
                vt_chunk(k, 0, 1, nc.scalar)
            vt_chunk(2, 1, 2, nc.gpsimd)
            vt_chunk(3, 1, 2, nc.gpsimd)
            bias_sb = persist.tile([128, bias_cols], f32, tag="bias")
            nc.gpsimd.dma_start(out=bias_sb[:], in_=bias_d.ap())
            sel_sb = persist.tile([128, N_MT * 32], bf16, tag="sel")
            nc.gpsimd.dma_start(out=sel_sb[:], in_=sel_d.ap())
            vt_chunk(0, 1, 2, nc.scalar)
            vt_chunk(1, 1, 2, nc.scalar)
            # warm the ACT spline tables (exp) while the first einsum runs:
            # the auto-inserted table load lands before this dep-free dummy
            warm = persist.tile([128, 1], f32, tag="warm")
            nc.scalar.activation(warm[:], nc.const_aps.tensor(0.0, (128, 1)),
                                 EXP, bias=0.0, scale=1.0)
            if M_ROWS > tt_head:
                for k in range(N_KC):
                    nc.sync.dma_start(
                        out=tt_tiles[k][:, tt_head:M_ROWS],
                        in_=tT_d.ap()[128 * k:128 * (k + 1), tt_head:M_ROWS])
            vt_chunk(0, 2, 4, nc.scalar)
            vt_chunk(1, 2, 4, nc.scalar)
            vt_chunk(2, 2, 4, nc.gpsimd)
            vt_chunk(3, 2, 4, nc.gpsimd)
            for k in range(N_KC):
                vt_chunk(k, 4, N_NT, nc.gpsimd)
        else:
            # v7/v8.0 DMA order: SP/Pool alternation, per-n single transfers
            _dma_q = [nc.sync, nc.gpsimd]
            tt_head = min(512, M_ROWS)
            for k in range(N_KC):
                _dma_q[0].dma_start(out=tt_tiles[k][:, 0:tt_head],
                                    in_=tT_d.ap()[128 * k:128 * (k + 1),
                                                  0:tt_head])
                vt_chunk(k, 0, 1, _dma_q[1])
            for k in range(N_KC):
                vt_chunk(k, 1, 2, _dma_q[k % 2])
            bias_sb = persist.tile([128, bias_cols], f32, tag="bias")
            nc.sync.dma_start(out=bias_sb[:], in_=bias_d.ap())
            sel_sb = persist.tile([128, N_MT * 32], bf16, tag="sel")
            nc.sync.dma_start(out=sel_sb[:], in_=sel_d.ap())
            for k in range(N_KC):
                vt_chunk(k, 2, 3, _dma_q[k % 2])
            for k in range(N_KC):
                vt_chunk(k, 3, 4, _dma_q[k % 2])
            if M_ROWS > tt_head:
                for k in range(N_KC):
                    _dma_q[k % 2].dma_start(
                        out=tt_tiles[k][:, tt_head:M_ROWS],
                        in_=tT_d.ap()[128 * k:128 * (k + 1), tt_head:M_ROWS])
            for n in range(4, N_NT):
                for k in range(N_KC):
                    vt_chunk(k, n, n + 1, _dma_q[k % 2])

        # S|N accumulator, side-major: col = side*(256*N_MT) + m*256 + n*32 + b
        sn_all = persist.tile([128, 2 * N_MT * 256], f32, tag="sn_all")
        # final staging, (j,q)-partition layout: row = 32j+q, col = 32h+b'
        # (video index = 128h + 32j + b'); avoids any partition-crossing DMA
        # until the single strided output DMA
        vt2_full = persist.tile([128, 64], f32, tag="vt2_full")
        bias0 = bias_sb[:, 0:1]

        def issue_sel(den2, num2, item):
            """The 4 over-t selector matmuls for one queued pair-tile.
            32-wide stationary: band j of den2/num2 is its own region-scoped
            accumulation group, complete at (mp==N_PAIR-1, mi==1)."""
            j, mp, exe = item
            for mi in range(2):
                m = 2 * mp + mi
                selm = sel_sb[:, 32 * m:32 * (m + 1)]
                band = slice(32 * j, 32 * (j + 1))
                nc.tensor.matmul(
                    den2[band, :], selm, exe[:, 512 * mi:512 * (mi + 1)],
                    start=(mp == 0 and mi == 0), stop=(mp == N_PAIR - 1 and mi == 1),
                    skip_group_check=True, tile_position=(0, 32 * j),
                )
                nc.tensor.matmul(
                    num2[band, :], selm, exe[:, 1024 + 512 * mi:1536 + 512 * mi],
                    start=(mp == 0 and mi == 0), stop=(mp == N_PAIR - 1 and mi == 1),
                    skip_group_check=True, tile_position=(0, 32 * j),
                )

        for _rep in range(reps):
            # t2v second-level accumulators in the (j,q)/(h,b') layout:
            # row = 32j+q, col = 32h+b'
            den3 = dn3_pool.tile([128, 64], f32, tag="den3")
            num3 = dn3_pool.tile([128, 64], f32, tag="num3")

            # --- phase-2 (t2v second level) for one m-tile and one b-half;
            # interleaved into the main loop right where its inputs are
            # ready.  Split into an a-stage (DVE ratio + ACT exp) and a
            # b-stage (DVE weight product + PE matmuls) so tail emission can
            # interleave several chains without idling DVE on ACT hops.
            def t2v_level2_a(m, h):
                mb, hb = m * 256, 128 * h
                s1 = sn_all[:, mb + hb:mb + hb + 128]
                n1 = sn_all[:, 256 * N_MT + mb + hb:256 * N_MT + mb + hb + 128]
                rs1 = t2v_pool.tile([128, 128], f32, tag="rs1")
                nc.vector.reciprocal(rs1[:], s1)
                t2v_t = t2v_pool.tile([128, 128], f32, tag="t2v_t")
                nc.vector.tensor_tensor(t2v_t[:], n1, rs1[:], op=MUL)
                w_t = w_pool.tile([128, 128], bf16, tag="w_t")
                nc.scalar.activation(w_t[:], t2v_t[:], EXP, bias=bias0,
                                     scale=TAU2)
                return t2v_t, w_t

            def t2v_level2_b(m, h, t2v_t, w_t):
                xw_t = w_pool.tile([128, 128], bf16, tag="xw_t")
                with nc.allow_low_precision(reason="bf16 softmax weights"):
                    nc.vector.tensor_tensor(xw_t[:], t2v_t[:], w_t[:], op=MUL)
                selm = sel_sb[:, 32 * m:32 * (m + 1)]
                for j in range(4):
                    band = slice(32 * j, 32 * (j + 1))
                    hcol = slice(32 * h, 32 * h + 32)
                    wcol = slice(32 * j, 32 * (j + 1))
                    nc.tensor.matmul(
                        den3[band, hcol], selm, w_t[:, wcol],
                        start=(m == 0), stop=(m == N_MT - 1),
                        skip_group_check=True, tile_position=(0, 32 * j),
                    )
                    nc.tensor.matmul(
                        num3[band, hcol], selm, xw_t[:, wcol],
                        start=(m == 0), stop=(m == N_MT - 1),
                        skip_group_check=True, tile_position=(0, 32 * j),
                    )

            def t2v_level2(m, h):
                t2v_t, w_t = t2v_level2_a(m, h)
                t2v_level2_b(m, h, t2v_t, w_t)

            # ---- main loop: halves (b 0:128 / 128:256) x M-pairs x N-tiles
            for h in range(2):
                den2 = dn2_pool.tile([128, 512], f32, tag="den2")
                num2 = dn2_pool.tile([128, 512], f32, tag="num2")
                sel_queue = []   # software pipeline: sel MMs run 2 tiles late
                for mp in range(N_PAIR):
                    for j in range(4):
                        n = 4 * h + j
                        # drain queued selector matmuls BEFORE this tile's
                        # einsum: their deps are >=1 tile old (>=4 normally),
                        # so they slot into the PE stream without waiting,
                        # and the last pair drains 2/tile so nearly nothing
                        # serializes after the final XE.
                        if _TUNE["pre_pop"]:
                            npop = 2 if mp == N_PAIR - 1 else (
                                1 if len(sel_queue) > 4 else 0)
                            for _ in range(npop):
                                if sel_queue:
                                    issue_sel(den2, num2, sel_queue.pop(0))
                        ps = ps_pool.tile([128, 1024], f32, tag="ps")
                        for mi in range(2):
                            m = 2 * mp + mi
                            for k in range(N_KC):
                                nc.tensor.matmul(
                                    ps[:, 512 * mi:512 * (mi + 1)],
                                    tt_tiles[k][:, 128 * m:128 * (m + 1)],
                                    vt_tiles[(k, n)],
                                    start=(k == 0),
                                    stop=(k == N_KC - 1),
                                )
                        # E|XE for the pair: [E(m0) E(m1) XE(m0) XE(m1)]
                        exe = e_pool.tile([128, 2048], bf16, tag="exe")
                        # the last pair of the half gates the v2t second
                        # level: split exp/mult per m-tile and multiply
                        # straight from PSUM (1x mode) so its selector
                        # matmuls issue ~1us earlier
                        tail_pair = (mp == N_PAIR - 1 and j == 3)
                        with nc.allow_low_precision(reason="bf16 E/XE"):
                            if tail_pair:
                                # both exps emitted adjacently: keeps the
                                # scheduler's coalesced ACT->PE wait from
                                # covering the selector matmuls in between
                                for mi in range(2):
                                    ms = slice(512 * mi, 512 * mi + 512)
                                    nc.scalar.activation(
                                        exe[:, ms], ps[:, ms], EXP,
                                        bias=bias0, scale=TAU)
                                for mi in range(2):
                                    ms = slice(512 * mi, 512 * mi + 512)
                                    xs = slice(1024 + 512 * mi,
                                               1536 + 512 * mi)
                                    nc.vector.scalar_tensor_tensor(
                                        out=exe[:, xs], in0=ps[:, ms],
                                        scalar=0.5, in1=exe[:, ms],
                                        op0=MUL, op1=MUL)
                            else:
                                nc.scalar.activation(
                                    exe[:, 0:1024], ps[:], EXP, bias=bias0,
                                    scale=TAU)
                                xbt = xb_pool.tile([128, 1024], bf16,
                                                   tag="xbt")
                                # the X-copy carries the global 0.5 fold
                                nc.scalar.activation(xbt[:], ps[:], CPY,
                                                     bias=0.0, scale=0.5)
                                nc.vector.tensor_tensor(
                                    exe[:, 1024:2048], xbt[:], exe[:, 0:1024],
                                    op=MUL)
                        # queue the 4 selector matmuls (over-t sums on PE),
                        # issued a few tiles later so their ACT/DVE deps are
                        # done by the time PE reaches them.  In the last pair
                        # the queue drains 2/tile so den2/num2 close right
                        # after the final XE instead of 4 items later.
                        sel_queue.append((j, mp, exe))
                        if _TUNE["pre_pop"]:
                            if tail_pair:
                                while sel_queue:
                                    issue_sel(den2, num2, sel_queue.pop(0))
                        elif _TUNE["flush2"] and mp == N_PAIR - 1:
                            for _ in range(2):
                                if sel_queue:
                                    issue_sel(den2, num2, sel_queue.pop(0))
                        elif len(sel_queue) > 4 or tail_pair:
                            issue_sel(den2, num2, sel_queue.pop(0))
                        # sum over v: 4-step strided bf16 add-tree on DVE.
                        # tr: [0:1024]=(c4,b32,v8)  [1024:1536]=(c,b,4)
                        #     [1536:1792]=(c,b,2)
                        tr = tr_pool.tile([128, 1792], bf16, tag="tr")
                        ev16 = exe[:].rearrange("p (c b v) -> p c b v", c=4,
                                                v=16)
                        t8 = tr[:, 0:1024].rearrange("p (c b v) -> p c b v",
                                                     c=4, v=8)
                        t4 = tr[:, 1024:1536].rearrange("p (c b v) -> p c b v",
                                                        c=4, v=4)
                        t2 = tr[:, 1536:1792].rearrange("p (c b v) -> p c b v",
                                                        c=4, v=2)
                        with nc.allow_low_precision(reason="bf16 v-tree"):
                            nc.vector.tensor_tensor(
                                t8[:], ev16[:, :, :, 0:8], ev16[:, :, :, 8:16],
                                op=ADD)
                            nc.vector.tensor_tensor(
                                t4[:], t8[:, :, :, 0:4], t8[:, :, :, 4:8],
                                op=ADD)
                            nc.vector.tensor_tensor(
                                t2[:], t4[:, :, :, 0:2], t4[:, :, :, 2:4],
                                op=ADD)
                        # final add writes fp32 S|N: col = s*(256*N_MT)+m*256+n*32+b
                        snv = sn_all[:].rearrange(
                            "p (s m nb) -> p s m nb", s=2, m=N_MT)[
                            :, :, 2 * mp:2 * mp + 2, 32 * n:32 * (n + 1)]
                        nc.vector.tensor_tensor(
                            snv, t2[:, :, :, 0:1], t2[:, :, :, 1:2], op=ADD)
                    # this pair's half-columns are complete: run its t2v
                    # second level overlapped with the remaining pairs (the
                    # last pair's runs after the v2t chain below, which is
                    # the longer critical path at the end of the half)
                    if mp < N_PAIR - 1:
                        t2v_level2(2 * mp, h)
                        t2v_level2(2 * mp + 1, h)
                while sel_queue:
                    issue_sel(den2, num2, sel_queue.pop(0))

                # ---- second level of v2t for this half (softmax over v),
                # split into two column groups (b' 0:16 / 16:32) so the serial
                # chain pipelines and the last-half tail is ~2x shorter ----
                v_t = v_pool.tile([128, 512], f32, tag="v_t")
                exev = v_pool.tile([128, 1024], bf16, tag="exev")
                snv_t = v_pool.tile([128, 64], f32, tag="snv_t")
                ev16 = exev[:].rearrange("p (c b v) -> p c b v", c=2, v=16)
                sn8 = snv_t[:].rearrange("p (c b) -> p c b", c=2)
                # ACT (idle here) evacuates den2/num2 from PSUM so the DVE
                # ratio is one SBUF-SBUF divide instead of a PSUM reciprocal
                # + PSUM multiply: ~0.9us less tail-DVE per half
                den2c = v_pool.tile([128, 512], f32, tag="den2c")
                num2c = v_pool.tile([128, 512], f32, tag="num2c")
                nc.scalar.activation(den2c[:], den2[:], CPY)
                nc.scalar.activation(num2c[:], num2[:], CPY)

                rden2 = v_pool.tile([128, 512], f32, tag="rden2")

                def v2t_a(cg):
                    cs = slice(256 * cg, 256 * cg + 256)
                    nc.vector.reciprocal(rden2[:, cs], den2c[:, cs])
                    nc.vector.tensor_tensor(
                        v_t[:, cs], num2c[:, cs], rden2[:, cs], op=MUL)
                    nc.scalar.activation(
                        exev[:, cs], v_t[:, cs], EXP, bias=bias0, scale=TAU2)

                def v2t_b(cg):
                    cs = slice(256 * cg, 256 * cg + 256)
                    bs = slice(16 * cg, 16 * cg + 16)
                    with nc.allow_low_precision(reason="bf16 Ev/XEv"):
                        nc.vector.tensor_tensor(
                            exev[:, 512 + 256 * cg:768 + 256 * cg],
                            v_t[:, cs], exev[:, cs], op=MUL)
                    # single reduce (no fast modes either way) instead of an
                    # add-tree: 1 instruction of latency on the tail chain
                    nc.vector.reduce_sum(
                        out=sn8[:, :, bs],
                        in_=ev16[:, :, bs, :],
                        axis=AX)
                    # vt2 = Nv'/Sv (the final /2 arrived via the XE fold),
                    # written straight into the (j,q)/(h,b') staging tile
                    rsv_t = v_pool.tile([128, 32], f32, tag="rsv_t")
                    nc.vector.reciprocal(rsv_t[:, bs], snv_t[:, bs])
                    nc.vector.tensor_tensor(
                        vt2_full[:, 32 * h + 16 * cg:32 * h + 16 * cg + 16],
                        snv_t[:, 32:64][:, bs], rsv_t[:, bs], op=MUL)

                # interleave the v2t chains, the deferred last pair's t2v
                # level 2, and phase 3 so DVE always has ready work while
                # ACT exps round-trip
                m_a, m_b = 2 * (N_PAIR - 1), 2 * N_PAIR - 1
                skip = _TUNE["skip"] if h == 1 else ""
                if "vb" in skip:
                    def v2t_b(cg):
                        pass
                if "va" in skip:
                    def v2t_a(cg):
                        pass
                if "lvl2" in skip:
                    def t2v_level2_a(m, h):
                        return None, None
                    def t2v_level2_b(m, h, *a):
                        pass
                if _TUNE["tail_order"] == 0:
                    v2t_a(0)
                    ta = t2v_level2_a(m_a, h)
                    tb = t2v_level2_a(m_b, h)
                    v2t_a(1)
                    v2t_b(0)
                    t2v_level2_b(m_a, h, *ta)
                    t2v_level2_b(m_b, h, *tb)
                    v2t_b(1)
                elif _TUNE["tail_order"] == 1:
                    # lvl2 chains first: their sn_all deps close before den2
                    ta = t2v_level2_a(m_a, h)
                    tb = t2v_level2_a(m_b, h)
                    v2t_a(0)
                    t2v_level2_b(m_a, h, *ta)
                    v2t_a(1)
                    t2v_level2_b(m_b, h, *tb)
                    v2t_b(0)
                    v2t_b(1)
                else:
                    ta = t2v_level2_a(m_a, h)
                    v2t_a(0)
                    tb = t2v_level2_a(m_b, h)
                    v2t_a(1)
                    t2v_level2_b(m_a, h, *ta)
                    v2t_b(0)
                    t2v_level2_b(m_b, h, *tb)
                    v2t_b(1)

                # ---- phase 3 for this half: t2v2 = 0.5*Num3/Den3, combine
                # with v2t, and DMA out[q, 128h+32j+b'].  den3/num3's h-region
                # groups stop at m==N_MT-1 just above, so the h=0 finale fully
                # overlaps the h=1 main loop. ----
                if "p3" in skip:
                    continue
                hcol = slice(32 * h, 32 * h + 32)
                rden3 = t2v_pool.tile([128, 32], f32, tag="rden3")
                nc.vector.reciprocal(rden3[:], den3[:, hcol])
                t2v2 = t2v_pool.tile([128, 32], f32, tag="t2v2")
                nc.vector.tensor_tensor(t2v2[:], num3[:, hcol], rden3[:],
                                        op=MUL)
                out_sb = t2v_pool.tile([128, 32], f32, tag="out_sb")
                nc.vector.tensor_tensor(out_sb[:], t2v2[:],
                                        vt2_full[:, hcol], op=ADD)
                out_ap = out_d.ap().rearrange(
                    "q (h j b) -> h j q b", h=2, j=4)[h:h + 1]
                nc.sync.dma_start(out=out_ap, in_=out_sb[:])

    nc.compile()
    return nc


def _get_program(n_pairs=3, reps=1, variant=0, **_ignored):
    key = (n_pairs, reps, variant)
    if key not in _PROGRAM_CACHE:
        _PROGRAM_CACHE[key] = _build_program(n_pairs, reps, variant)
    return _PROGRAM_CACHE[key]


def _l2norm(a):
    n = np.linalg.norm(a, axis=-1, keepdims=True)
    return a / np.maximum(n, EPS)


def _bf16(a):
    import ml_dtypes
    return a.astype(ml_dtypes.bfloat16)


def _assign_queries(lengths):
    """Greedy LPT: partition the A queries into N_CORES groups of A_LOC,
    minimizing the max total token count per core."""
    order = np.argsort(-lengths, kind="stable")
    core_sum = np.zeros(N_CORES, np.int64)
    core_cnt = np.zeros(N_CORES, np.int64)
    groups = [[] for _ in range(N_CORES)]
    for q in order:
        open_c = np.nonzero(core_cnt < A_LOC)[0]
        c = open_c[np.argmin(core_sum[open_c])]
        groups[c].append(int(q))
        core_sum[c] += lengths[q]
        core_cnt[c] += 1
    return groups, int(core_sum.max())


def prepare_inputs(text_feat, video_feat, text_mask):
    """Host-side shard/layout prep with valid-token compaction.
    Returns (in_maps for the 8 cores, query permutation, n_pairs)."""
    t = _l2norm(text_feat.astype(np.float32))          # [A, T, D]
    v = _l2norm(video_feat.astype(np.float32))         # [B, V, D]
    mask = text_mask.astype(np.float32)

    # bake mask values into the text rows: logits = (mask*t) . v exactly
    tm = t * mask[:, :, None]

    lengths = (mask != 0.0).sum(axis=1).astype(np.int64)   # valid tokens per query
    groups, max_rows = _assign_queries(lengths)
    n_pairs = max(1, -(-max_rows // 256))                   # ceil to 256-row pairs
    m_rows = 256 * n_pairs
    n_mt = 2 * n_pairs

    # video: [B, V, D] -> [D, B*V], shared by all cores
    vT = _bf16(np.ascontiguousarray(v.reshape(B * V_FRM, D).T))

    in_maps = []
    perm = np.empty(A, np.int64)
    for c in range(N_CORES):
        qs = groups[c]
        perm[c * A_LOC:(c + 1) * A_LOC] = qs
        rows = np.zeros((m_rows, D), np.float32)
        sel = np.zeros((128, n_mt * 32), np.float32)
        r = 0
        for jloc, q in enumerate(qs):
            idx = np.nonzero(mask[q] != 0.0)[0]
            k = len(idx)
            rows[r:r + k] = tm[q, idx]
            rr = np.arange(r, r + k)
            sel[rr % 128, (rr // 128) * 32 + jloc] = 1.0
            r += k
        tT = _bf16(np.ascontiguousarray(rows.T))            # [D, m_rows]
        bias = np.full((128, 1), SHIFT, np.float32)
        in_maps.append({"tT": tT, "vT": vT, "sel": _bf16(sel), "bias": bias})
    return in_maps, perm, n_pairs


def run(in_maps, n_pairs=3, trace=False, reps=1, variant=0, **kwargs):
    import concourse.mybir as mybir
    from concourse import bass_utils

    nc = _get_program(n_pairs=n_pairs, reps=reps, variant=variant)
    # pad inputs to the program's declared shapes (bias width varies by build)
    shapes = {}
    for alloc in nc.m.functions[0].allocations:
        if isinstance(alloc, mybir.MemoryLocationSet) and alloc.kind == "ExternalInput":
            shapes[alloc.memorylocations[0].name] = tuple(alloc.tensor_shape)
    fixed = []
    for m in in_maps:
        mm = {}
        for k, v in m.items():
            shp = shapes.get(k, tuple(v.shape))
            if tuple(v.shape) != shp:
                out = np.full(shp, SHIFT if k == "bias" else 0.0, v.dtype)
                sl = tuple(slice(0, min(s, t)) for s, t in zip(v.shape, shp))
                out[sl] = v[sl]
                mm[k] = out
            else:
                mm[k] = v
        fixed.append(mm)
    return bass_utils.run_bass_kernel_spmd(
        nc, fixed, core_ids=list(range(N_CORES)), trace=trace, **kwargs
    )


def kernel(text_feat, video_feat, text_mask):
    in_maps, perm, n_pairs = prepare_inputs(
        np.asarray(text_feat), np.asarray(video_feat), np.asarray(text_mask)
    )
    res = run(in_maps, n_pairs=n_pairs)
    packed = np.concatenate([res.results[c]["out"] for c in range(N_CORES)], axis=0)
    out = np.empty_like(packed)
    out[perm] = packed                      # un-permute queries to original order
    return out.astype(np.float32)


# revision 37
# speedup vs baseline: 1.1060x; 1.0234x over previous
"""Trainium2 Bass kernel for the two-level softmax-pooled text/video retrieval head.

Computes, for text_feat [256,32,512], video_feat [256,16,512], text_mask [256,32]:
    out[a,b] = (t2v(a,b) + v2t(a,b)) / 2
where t2v/v2t are two-level softmax-weighted poolings of the cross token/frame
cosine similarity tensor logits[a,b,t,v] (see reference module).

Sharding: text axis A split across 8 NeuronCores (32 queries each); video
features replicated. Host does l2-normalization + transposition (layout prep);
the device does all einsum + softmax compute.

v8 design: on top of the v7 pipeline (see below), VALID-TOKEN COMPACTION.
Lengths are in [16,32) (mean ~23.4), so ~27% of the (q,t) rows are padding
that v7 computed and then masked away via the 0/1 selector values.  v8
instead packs only the valid tokens of each query into consecutive rows and
lets the (already data-dependent) selector matmuls carry the arbitrary
row->query map.  Queries are assigned to cores by greedy LPT balancing so
every core's row count fits the same SPMD program (749 rows max vs the 768
capacity of 3 m-pairs for the reference inputs; a 4-pair program is compiled
as fallback if some other input needs it).  This cuts the main loop from 32
to 24 (pair,n) tiles: einsum, exp/copy, XE, v-tree and selector work all
drop ~25% across PE/ACT/DVE.  Mask values are baked into the text features
(t_row *= mask) so non-binary masks would still be exact.

v7 pipeline (per core, N_PAIR=3: 768 (q,t) rows; B=256, V=16 -> 4096 (b,v)
cols; D=512):
  - inputs are bf16 (0.4% feature rounding, ~1.7% first-level softmax weight
    noise -- well inside the 2e-2 tolerance); halves input DMA.
  - m-tiles are processed in PAIRS: one [128,1024] PSUM tile (2 banks) per
    pair, so ACT exp / ACT X-copy / DVE XE-mult each run at [128,1024] width.
  - E and XE are bf16; the XE multiply runs in the DVE 2x_1p mode.
  - sum-over-v (t2v level 1) is a 4-step strided add-tree on DVE (bf16 2x).
  - sum-over-t (v2t level 1 den/num) stays on PE as selector matmuls (bf16,
    exact 0/1, 32-wide stationary -> region-scoped PSUM accumulation),
    software-pipelined behind the main matmuls.
  - loop order is pair-outer / n-tile-inner within each b-half, so the t2v
    second level for a pair runs as soon as its half's columns are done.
"""

import sys

import numpy as np

if "/opt/trn_rl_repo" not in sys.path:
    sys.path.insert(0, "/opt/trn_rl_repo")

A, T_TOK, B, V_FRM, D = 256, 32, 256, 16, 512
N_CORES = 8
A_LOC = A // N_CORES            # 32 queries per core
N_COLS = B * V_FRM              # 4096  (b,v) cols
N_KC = D // 128                 # 4 K-chunks
N_NT = N_COLS // 512            # 8 N-tiles (32 videos each)
TAU = 100.0
SHIFT = -30.0                   # global exp shift (softmax-invariant)
EPS = 1e-6

_PROGRAM_CACHE = {}

# build-time scheduling knobs (A/B tuning; defaults = shipped config)
_TUNE = {"flush2": False, "dma_new": False, "pre_pop": False, "tail_order": 0,
         "e_bufs": 13, "skip": ""}


def _build_program(n_pairs=3, reps=1, variant=0):
    import contextlib

    import concourse.mybir as mybir
    import concourse.tile as tile
    from concourse import bacc

    N_PAIR = n_pairs
    N_MT = 2 * N_PAIR               # m-tiles of 128 rows
    M_ROWS = 128 * N_MT             # compacted (q,t) rows incl. padding

    f32 = mybir.dt.float32
    bf16 = mybir.dt.bfloat16
    EXP = mybir.ActivationFunctionType.Exp
    CPY = mybir.ActivationFunctionType.Copy
    MUL = mybir.AluOpType.mult
    ADD = mybir.AluOpType.add
    DIV = mybir.AluOpType.divide
    AX = mybir.AxisListType.X
    # The final /2 is folded into XE at creation (X-copy scale 0.5), so every
    # numerator downstream is pre-halved and each num/den ratio site becomes a
    # single DVE divide; the softmax exps compensate with scale 2*TAU.
    TAU2 = 2.0 * TAU

    nc = bacc.Bacc("TRN2", target_bir_lowering=False, debug=False)

    tT_d = nc.dram_tensor("tT", [D, M_ROWS], bf16, kind="ExternalInput")
    vT_d = nc.dram_tensor("vT", [D, N_COLS], bf16, kind="ExternalInput")
    sel_d = nc.dram_tensor("sel", [128, N_MT * 32], bf16, kind="ExternalInput")
    # bias width varies with reps/variant so each build gets a distinct HLO
    # hash (the NEFF cache otherwise silently reuses the first-compiled
    # program)
    bias_cols = 1 + (reps - 1) + 7 * variant
    bias_d = nc.dram_tensor("bias", [128, bias_cols], f32, kind="ExternalInput")
    out_d = nc.dram_tensor("out", [A_LOC, B], f32, kind="ExternalOutput")

    with tile.TileContext(nc) as tc, contextlib.ExitStack() as ctx:
        persist = ctx.enter_context(tc.tile_pool(name="persist", bufs=1))
        ps_pool = ctx.enter_context(tc.tile_pool(name="ps", bufs=2, space="PSUM"))
        dn2_pool = ctx.enter_context(tc.tile_pool(name="dn2", bufs=1, space="PSUM"))
        dn3_pool = ctx.enter_context(tc.tile_pool(name="dn3", bufs=1, space="PSUM"))
        e_pool = ctx.enter_context(tc.tile_pool(name="e", bufs=_TUNE["e_bufs"]))
        tr_pool = ctx.enter_context(tc.tile_pool(name="tr", bufs=3))
        xb_pool = ctx.enter_context(tc.tile_pool(name="xb", bufs=3))
        t2v_pool = ctx.enter_context(tc.tile_pool(name="t2v", bufs=2))
        w_pool = ctx.enter_context(tc.tile_pool(name="w", bufs=3))
        v_pool = ctx.enter_context(tc.tile_pool(name="v2", bufs=2))

        # ---- persistent inputs. Four issue queues in parallel (SP/DVE/ACT
        # HWDGE + Pool SWDGE) so descriptor generation (~0.6-1us per DMA) is
        # not the startup bottleneck.  Critical-path tiles (first pair's text
        # chunks, the n=0/1 video tiles) go out as small single transfers in
        # consumption order; everything later is batched into wide transfers.
        tt_tiles = []
        vt_tiles = {}
        for k in range(N_KC):
            t_ = persist.tile([128, M_ROWS], bf16, tag=f"tt_{k}")
            tt_tiles.append(t_)

        def vt_chunk(k, n0, n1, q):
            t_ = persist.tile([128, 512 * (n1 - n0)], bf16, tag=f"vt_{k}_{n0}")
            q.dma_start(out=t_[:],
                        in_=vT_d.ap()[128 * k:128 * (k + 1),
                                      512 * n0:512 * n1])
            for n in range(n0, n1):
                vt_tiles[(k, n)] = t_[:, 512 * (n - n0):512 * (n - n0 + 1)]

        if _TUNE["dma_new"]:
            # text heads (first pair) + first video tiles: smallest possible
            # critical path to the first matmul.  HWDGE queues: SP carries the
            # text, ACT carries the first-half video; the Pool SWDGE queue
            # takes the constants + late n-tiles.
            tt_head = min(256, M_ROWS)
            for k in range(N_KC):
                nc.sync.dma_start(out=tt_tiles[k][:, 0:tt_head],
                                  in_=tT_d.ap()[128 * k:128 * (k + 1),
                                                0:tt_head])
                vt_chunk(k, 0, 1, nc.scalar)
            vt_chunk(2, 1, 2, nc.gpsimd)
            vt_chunk(3, 1, 2, nc.gpsimd)
            bias_sb = persist.tile([128, bias_cols], f32, tag="bias")
            nc.gpsimd.dma_start(out=bias_sb[:], in_=bias_d.ap())
            sel_sb = persist.tile([128, N_MT * 32], bf16, tag="sel")
            nc.gpsimd.dma_start(out=sel_sb[:], in_=sel_d.ap())
            vt_chunk(0, 1, 2, nc.scalar)
            vt_chunk(1, 1, 2, nc.scalar)
            # warm the ACT spline tables (exp) while the first einsum runs:
            # the auto-inserted table load lands before this dep-free dummy
            warm = persist.tile([128, 1], f32, tag="warm")
            nc.scalar.activation(warm[:], nc.const_aps.tensor(0.0, (128, 1)),
                                 EXP, bias=0.0, scale=1.0)
            if M_ROWS > tt_head:
                for k in range(N_KC):
                    nc.sync.dma_start(
                        out=tt_tiles[k][:, tt_head:M_ROWS],
                        in_=tT_d.ap()[128 * k:128 * (k + 1), tt_head:M_ROWS])
            vt_chunk(0, 2, 4, nc.scalar)
            vt_chunk(1, 2, 4, nc.scalar)
            vt_chunk(2, 2, 4, nc.gpsimd)
            vt_chunk(3, 2, 4, nc.gpsimd)
            for k in range(N_KC):
                vt_chunk(k, 4, N_NT, nc.gpsimd)
        else:
            # v7/v8.0 DMA order: SP/Pool alternation, per-n single transfers
            _dma_q = [nc.sync, nc.gpsimd]
            tt_head = min(512, M_ROWS)
            for k in range(N_KC):
                _dma_q[0].dma_start(out=tt_tiles[k][:, 0:tt_head],
                                    in_=tT_d.ap()[128 * k:128 * (k + 1),
                                                  0:tt_head])
                vt_chunk(k, 0, 1, _dma_q[1])
            for k in range(N_KC):
                vt_chunk(k, 1, 2, _dma_q[k % 2])
            bias_sb = persist.tile([128, bias_cols], f32, tag="bias")
            nc.sync.dma_start(out=bias_sb[:], in_=bias_d.ap())
            sel_sb = persist.tile([128, N_MT * 32], bf16, tag="sel")
            nc.sync.dma_start(out=sel_sb[:], in_=sel_d.ap())
            if _TUNE.get("warm_act", True):
                # warm the ACT spline tables while the first einsum runs
                warm = persist.tile([128, 1], f32, tag="warm")
                nc.scalar.activation(warm[:],
                                     nc.const_aps.tensor(0.0, (128, 1)),
                                     EXP, bias=0.0, scale=1.0)
            for k in range(N_KC):
                vt_chunk(k, 2, 3, _dma_q[k % 2])
            for k in range(N_KC):
                vt_chunk(k, 3, 4, _dma_q[k % 2])
            if M_ROWS > tt_head:
                for k in range(N_KC):
                    _dma_q[k % 2].dma_start(
                        out=tt_tiles[k][:, tt_head:M_ROWS],
                        in_=tT_d.ap()[128 * k:128 * (k + 1), tt_head:M_ROWS])
            for n in range(4, N_NT):
                for k in range(N_KC):
                    vt_chunk(k, n, n + 1, _dma_q[k % 2])

        # S|N accumulator, side-major: col = side*(256*N_MT) + m*256 + n*32 + b
        sn_all = persist.tile([128, 2 * N_MT * 256], f32, tag="sn_all")
        # final staging, (j,q)-partition layout: row = 32j+q, col = 32h+b'
        # (video index = 128h + 32j + b'); avoids any partition-crossing DMA
        # until the single strided output DMA
        vt2_full = persist.tile([128, 64], f32, tag="vt2_full")
        bias0 = bias_sb[:, 0:1]

        def issue_sel(den2, num2, item):
            """The 4 over-t selector matmuls for one queued pair-tile.
            32-wide stationary: band j of den2/num2 is its own region-scoped
            accumulation group, complete at (mp==N_PAIR-1, mi==1)."""
            j, mp, exe = item
            for mi in range(2):
                m = 2 * mp + mi
                selm = sel_sb[:, 32 * m:32 * (m + 1)]
                band = slice(32 * j, 32 * (j + 1))
                nc.tensor.matmul(
                    den2[band, :], selm, exe[:, 512 * mi:512 * (mi + 1)],
                    start=(mp == 0 and mi == 0), stop=(mp == N_PAIR - 1 and mi == 1),
                    skip_group_check=True, tile_position=(0, 32 * j),
                )
                nc.tensor.matmul(
                    num2[band, :], selm, exe[:, 1024 + 512 * mi:1536 + 512 * mi],
                    start=(mp == 0 and mi == 0), stop=(mp == N_PAIR - 1 and mi == 1),
                    skip_group_check=True, tile_position=(0, 32 * j),
                )

        for _rep in range(reps):
            # t2v second-level accumulators in the (j,q)/(h,b') layout:
            # row = 32j+q, col = 32h+b'
            den3 = dn3_pool.tile([128, 64], f32, tag="den3")
            num3 = dn3_pool.tile([128, 64], f32, tag="num3")

            # --- phase-2 (t2v second level) for one m-tile and one b-half;
            # interleaved into the main loop right where its inputs are
            # ready.  Split into an a-stage (DVE ratio + ACT exp) and a
            # b-stage (DVE weight product + PE matmuls) so tail emission can
            # interleave several chains without idling DVE on ACT hops.
            def t2v_level2_a(m, h):
                mb, hb = m * 256, 128 * h
                s1 = sn_all[:, mb + hb:mb + hb + 128]
                n1 = sn_all[:, 256 * N_MT + mb + hb:256 * N_MT + mb + hb + 128]
                rs1 = t2v_pool.tile([128, 128], f32, tag="rs1")
                nc.vector.reciprocal(rs1[:], s1)
                t2v_t = t2v_pool.tile([128, 128], f32, tag="t2v_t")
                nc.vector.tensor_tensor(t2v_t[:], n1, rs1[:], op=MUL)
                w_t = w_pool.tile([128, 128], bf16, tag="w_t")
                nc.scalar.activation(w_t[:], t2v_t[:], EXP, bias=bias0,
                                     scale=TAU2)
                return t2v_t, w_t

            def t2v_level2_b(m, h, t2v_t, w_t):
                xw_t = w_pool.tile([128, 128], bf16, tag="xw_t")
                with nc.allow_low_precision(reason="bf16 softmax weights"):
                    nc.vector.tensor_tensor(xw_t[:], t2v_t[:], w_t[:], op=MUL)
                selm = sel_sb[:, 32 * m:32 * (m + 1)]
                for j in range(4):
                    band = slice(32 * j, 32 * (j + 1))
                    hcol = slice(32 * h, 32 * h + 32)
                    wcol = slice(32 * j, 32 * (j + 1))
                    nc.tensor.matmul(
                        den3[band, hcol], selm, w_t[:, wcol],
                        start=(m == 0), stop=(m == N_MT - 1),
                        skip_group_check=True, tile_position=(0, 32 * j),
                    )
                    nc.tensor.matmul(
                        num3[band, hcol], selm, xw_t[:, wcol],
                        start=(m == 0), stop=(m == N_MT - 1),
                        skip_group_check=True, tile_position=(0, 32 * j),
                    )

            def t2v_level2(m, h):
                t2v_t, w_t = t2v_level2_a(m, h)
                t2v_level2_b(m, h, t2v_t, w_t)

            # ---- main loop: halves (b 0:128 / 128:256) x M-pairs x N-tiles
            for h in range(2):
                den2 = dn2_pool.tile([128, 512], f32, tag="den2")
                num2 = dn2_pool.tile([128, 512], f32, tag="num2")
                sel_queue = []   # software pipeline: sel MMs run 2 tiles late
                for mp in range(N_PAIR):
                    for j in range(4):
                        n = 4 * h + j
                        # drain queued selector matmuls BEFORE this tile's
                        # einsum: their deps are >=1 tile old (>=4 normally),
                        # so they slot into the PE stream without waiting,
                        # and the last pair drains 2/tile so nearly nothing
                        # serializes after the final XE.
                        if _TUNE["pre_pop"]:
                            npop = 2 if mp == N_PAIR - 1 else (
                                1 if len(sel_queue) > 4 else 0)
                            for _ in range(npop):
                                if sel_queue:
                                    issue_sel(den2, num2, sel_queue.pop(0))
                        ps = ps_pool.tile([128, 1024], f32, tag="ps")
                        for mi in range(2):
                            m = 2 * mp + mi
                            for k in range(N_KC):
                                nc.tensor.matmul(
                                    ps[:, 512 * mi:512 * (mi + 1)],
                                    tt_tiles[k][:, 128 * m:128 * (m + 1)],
                                    vt_tiles[(k, n)],
                                    start=(k == 0),
                                    stop=(k == N_KC - 1),
                                )
                        # E|XE for the pair: [E(m0) E(m1) XE(m0) XE(m1)]
                        exe = e_pool.tile([128, 2048], bf16, tag="exe")
                        # the last pair of the half gates the v2t second
                        # level: split exp/mult per m-tile and multiply
                        # straight from PSUM (1x mode) so its selector
                        # matmuls issue ~1us earlier
                        tail_pair = (mp == N_PAIR - 1 and j == 3)
                        with nc.allow_low_precision(reason="bf16 E/XE"):
                            if tail_pair:
                                # both exps emitted adjacently: keeps the
                                # scheduler's coalesced ACT->PE wait from
                                # covering the selector matmuls in between
                                for mi in range(2):
                                    ms = slice(512 * mi, 512 * mi + 512)
                                    nc.scalar.activation(
                                        exe[:, ms], ps[:, ms], EXP,
                                        bias=bias0, scale=TAU)
                                for mi in range(2):
                                    ms = slice(512 * mi, 512 * mi + 512)
                                    xs = slice(1024 + 512 * mi,
                                               1536 + 512 * mi)
                                    nc.vector.scalar_tensor_tensor(
                                        out=exe[:, xs], in0=ps[:, ms],
                                        scalar=0.5, in1=exe[:, ms],
                                        op0=MUL, op1=MUL)
                            else:
                                nc.scalar.activation(
                                    exe[:, 0:1024], ps[:], EXP, bias=bias0,
                                    scale=TAU)
                                xbt = xb_pool.tile([128, 1024], bf16,
                                                   tag="xbt")
                                # the X-copy carries the global 0.5 fold
                                nc.scalar.activation(xbt[:], ps[:], CPY,
                                                     bias=0.0, scale=0.5)
                                nc.vector.tensor_tensor(
                                    exe[:, 1024:2048], xbt[:], exe[:, 0:1024],
                                    op=MUL)
                        # queue the 4 selector matmuls (over-t sums on PE),
                        # issued a few tiles later so their ACT/DVE deps are
                        # done by the time PE reaches them.  In the last pair
                        # the queue drains 2/tile so den2/num2 close right
                        # after the final XE instead of 4 items later.
                        sel_queue.append((j, mp, exe))
                        if _TUNE["pre_pop"]:
                            if tail_pair:
                                while sel_queue:
                                    issue_sel(den2, num2, sel_queue.pop(0))
                        elif _TUNE["flush2"] and mp == N_PAIR - 1:
                            for _ in range(2):
                                if sel_queue:
                                    issue_sel(den2, num2, sel_queue.pop(0))
                        elif len(sel_queue) > 4 or tail_pair:
                            issue_sel(den2, num2, sel_queue.pop(0))
                        # sum over v: 4-step strided bf16 add-tree on DVE.
                        # tr: [0:1024]=(c4,b32,v8)  [1024:1536]=(c,b,4)
                        #     [1536:1792]=(c,b,2)
                        tr = tr_pool.tile([128, 1792], bf16, tag="tr")
                        ev16 = exe[:].rearrange("p (c b v) -> p c b v", c=4,
                                                v=16)
                        t8 = tr[:, 0:1024].rearrange("p (c b v) -> p c b v",
                                                     c=4, v=8)
                        t4 = tr[:, 1024:1536].rearrange("p (c b v) -> p c b v",
                                                        c=4, v=4)
                        t2 = tr[:, 1536:1792].rearrange("p (c b v) -> p c b v",
                                                        c=4, v=2)
                        with nc.allow_low_precision(reason="bf16 v-tree"):
                            nc.vector.tensor_tensor(
                                t8[:], ev16[:, :, :, 0:8], ev16[:, :, :, 8:16],
                                op=ADD)
                            nc.vector.tensor_tensor(
                                t4[:], t8[:, :, :, 0:4], t8[:, :, :, 4:8],
                                op=ADD)
                            nc.vector.tensor_tensor(
                                t2[:], t4[:, :, :, 0:2], t4[:, :, :, 2:4],
                                op=ADD)
                        # final add writes fp32 S|N: col = s*(256*N_MT)+m*256+n*32+b
                        snv = sn_all[:].rearrange(
                            "p (s m nb) -> p s m nb", s=2, m=N_MT)[
                            :, :, 2 * mp:2 * mp + 2, 32 * n:32 * (n + 1)]
                        nc.vector.tensor_tensor(
                            snv, t2[:, :, :, 0:1], t2[:, :, :, 1:2], op=ADD)
                    # this pair's half-columns are complete: run its t2v
                    # second level overlapped with the remaining pairs (the
                    # last pair's runs after the v2t chain below, which is
                    # the longer critical path at the end of the half)
                    if mp < N_PAIR - 1:
                        t2v_level2(2 * mp, h)
                        t2v_level2(2 * mp + 1, h)
                while sel_queue:
                    issue_sel(den2, num2, sel_queue.pop(0))

                # ---- second level of v2t for this half (softmax over v),
                # split into two column groups (b' 0:16 / 16:32) so the serial
                # chain pipelines and the last-half tail is ~2x shorter ----
                v_t = v_pool.tile([128, 512], f32, tag="v_t")
                exev = v_pool.tile([128, 1024], bf16, tag="exev")
                snv_t = v_pool.tile([128, 64], f32, tag="snv_t")
                ev16 = exev[:].rearrange("p (c b v) -> p c b v", c=2, v=16)
                sn8 = snv_t[:].rearrange("p (c b) -> p c b", c=2)
                # ACT (idle here) can evacuate den2/num2 from PSUM so the DVE
                # reciprocal+multiply read SBUF (no PSUM access penalty)
                if _TUNE.get("evac", False):
                    den2c = v_pool.tile([128, 512], f32, tag="den2c")
                    num2c = v_pool.tile([128, 512], f32, tag="num2c")
                    nc.scalar.activation(den2c[:], den2[:], CPY)
                    nc.scalar.activation(num2c[:], num2[:], CPY)
                else:
                    den2c, num2c = den2, num2

                rden2 = v_pool.tile([128, 512], f32, tag="rden2")

                def v2t_a(cg):
                    cs = slice(256 * cg, 256 * cg + 256)
                    nc.vector.reciprocal(rden2[:, cs], den2c[:, cs])
                    nc.vector.tensor_tensor(
                        v_t[:, cs], num2c[:, cs], rden2[:, cs], op=MUL)
                    nc.scalar.activation(
                        exev[:, cs], v_t[:, cs], EXP, bias=bias0, scale=TAU2)

                def v2t_b(cg):
                    cs = slice(256 * cg, 256 * cg + 256)
                    bs = slice(16 * cg, 16 * cg + 16)
                    with nc.allow_low_precision(reason="bf16 Ev/XEv"):
                        nc.vector.tensor_tensor(
                            exev[:, 512 + 256 * cg:768 + 256 * cg],
                            v_t[:, cs], exev[:, cs], op=MUL)
                    # single reduce (no fast modes either way) instead of an
                    # add-tree: 1 instruction of latency on the tail chain.
                    # Optionally on the (idle) Pool engine to relieve DVE,
                    # which is the throughput bound of this tail.
                    red_eng = nc.gpsimd if _TUNE.get("pool_red", False) \
                        else nc.vector
                    red_eng.reduce_sum(
                        out=sn8[:, :, bs],
                        in_=ev16[:, :, bs, :],
                        axis=AX)
                    # vt2 = Nv'/Sv (the final /2 arrived via the XE fold),
                    # written straight into the (j,q)/(h,b') staging tile
                    rsv_t = v_pool.tile([128, 32], f32, tag="rsv_t")
                    nc.vector.reciprocal(rsv_t[:, bs], snv_t[:, bs])
                    nc.vector.tensor_tensor(
                        vt2_full[:, 32 * h + 16 * cg:32 * h + 16 * cg + 16],
                        snv_t[:, 32:64][:, bs], rsv_t[:, bs], op=MUL)

                # interleave the v2t chains, the deferred last pair's t2v
                # level 2, and phase 3 so DVE always has ready work while
                # ACT exps round-trip
                m_a, m_b = 2 * (N_PAIR - 1), 2 * N_PAIR - 1
                skip = _TUNE["skip"] if h == 1 else ""
                if "vb" in skip:
                    def v2t_b(cg):
                        pass
                if "va" in skip:
                    def v2t_a(cg):
                        pass
                if "lvl2" in skip:
                    def t2v_level2_a(m, h):
                        return None, None
                    def t2v_level2_b(m, h, *a):
                        pass
                if _TUNE["tail_order"] == 0:
                    v2t_a(0)
                    ta = t2v_level2_a(m_a, h)
                    tb = t2v_level2_a(m_b, h)
                    v2t_a(1)
                    v2t_b(0)
                    t2v_level2_b(m_a, h, *ta)
                    t2v_level2_b(m_b, h, *tb)
                    v2t_b(1)
                elif _TUNE["tail_order"] == 1:
                    # lvl2 chains first: their sn_all deps close before den2
                    ta = t2v_level2_a(m_a, h)
                    tb = t2v_level2_a(m_b, h)
                    v2t_a(0)
                    t2v_level2_b(m_a, h, *ta)
                    v2t_a(1)
                    t2v_level2_b(m_b, h, *tb)
                    v2t_b(0)
                    v2t_b(1)
                else:
                    ta = t2v_level2_a(m_a, h)
                    v2t_a(0)
                    tb = t2v_level2_a(m_b, h)
                    v2t_a(1)
                    t2v_level2_b(m_a, h, *ta)
                    v2t_b(0)
                    t2v_level2_b(m_b, h, *tb)
                    v2t_b(1)

                # ---- phase 3 for this half: t2v2 = 0.5*Num3/Den3, combine
                # with v2t, and DMA out[q, 128h+32j+b'].  den3/num3's h-region
                # groups stop at m==N_MT-1 just above, so the h=0 finale fully
                # overlaps the h=1 main loop. ----
                if "p3" in skip:
                    continue
                hcol = slice(32 * h, 32 * h + 32)
                rden3 = t2v_pool.tile([128, 32], f32, tag="rden3")
                nc.vector.reciprocal(rden3[:], den3[:, hcol])
                t2v2 = t2v_pool.tile([128, 32], f32, tag="t2v2")
                nc.vector.tensor_tensor(t2v2[:], num3[:, hcol], rden3[:],
                                        op=MUL)
                out_sb = t2v_pool.tile([128, 32], f32, tag="out_sb")
                nc.vector.tensor_tensor(out_sb[:], t2v2[:],
                                        vt2_full[:, hcol], op=ADD)
                out_ap = out_d.ap().rearrange(
                    "q (h j b) -> h j q b", h=2, j=4)[h:h + 1]
                nc.sync.dma_start(out=out_ap, in_=out_sb[:])

    nc.compile()
    return nc


def _get_program(n_pairs=3, reps=1, variant=0, **_ignored):
    key = (n_pairs, reps, variant)
    if key not in _PROGRAM_CACHE:
        _PROGRAM_CACHE[key] = _build_program(n_pairs, reps, variant)
    return _PROGRAM_CACHE[key]


def _l2norm(a):
    n = np.linalg.norm(a, axis=-1, keepdims=True)
    return a / np.maximum(n, EPS)


def _bf16(a):
    import ml_dtypes
    return a.astype(ml_dtypes.bfloat16)


def _assign_queries(lengths):
    """Greedy LPT: partition the A queries into N_CORES groups of A_LOC,
    minimizing the max total token count per core."""
    order = np.argsort(-lengths, kind="stable")
    core_sum = np.zeros(N_CORES, np.int64)
    core_cnt = np.zeros(N_CORES, np.int64)
    groups = [[] for _ in range(N_CORES)]
    for q in order:
        open_c = np.nonzero(core_cnt < A_LOC)[0]
        c = open_c[np.argmin(core_sum[open_c])]
        groups[c].append(int(q))
        core_sum[c] += lengths[q]
        core_cnt[c] += 1
    return groups, int(core_sum.max())


def prepare_inputs(text_feat, video_feat, text_mask):
    """Host-side shard/layout prep with valid-token compaction.
    Returns (in_maps for the 8 cores, query permutation, n_pairs)."""
    t = _l2norm(text_feat.astype(np.float32))          # [A, T, D]
    v = _l2norm(video_feat.astype(np.float32))         # [B, V, D]
    mask = text_mask.astype(np.float32)

    # bake mask values into the text rows: logits = (mask*t) . v exactly
    tm = t * mask[:, :, None]

    lengths = (mask != 0.0).sum(axis=1).astype(np.int64)   # valid tokens per query
    groups, max_rows = _assign_queries(lengths)
    n_pairs = max(1, -(-max_rows // 256))                   # ceil to 256-row pairs
    m_rows = 256 * n_pairs
    n_mt = 2 * n_pairs

    # video: [B, V, D] -> [D, B*V], shared by all cores
    vT = _bf16(np.ascontiguousarray(v.reshape(B * V_FRM, D).T))

    in_maps = []
    perm = np.empty(A, np.int64)
    for c in range(N_CORES):
        qs = groups[c]
        perm[c * A_LOC:(c + 1) * A_LOC] = qs
        rows = np.zeros((m_rows, D), np.float32)
        sel = np.zeros((128, n_mt * 32), np.float32)
        r = 0
        for jloc, q in enumerate(qs):
            idx = np.nonzero(mask[q] != 0.0)[0]
            k = len(idx)
            rows[r:r + k] = tm[q, idx]
            rr = np.arange(r, r + k)
            sel[rr % 128, (rr // 128) * 32 + jloc] = 1.0
            r += k
        tT = _bf16(np.ascontiguousarray(rows.T))            # [D, m_rows]
        bias = np.full((128, 1), SHIFT, np.float32)
        in_maps.append({"tT": tT, "vT": vT, "sel": _bf16(sel), "bias": bias})
    return in_maps, perm, n_pairs


def run(in_maps, n_pairs=3, trace=False, reps=1, variant=0, **kwargs):
    import concourse.mybir as mybir
    from concourse import bass_utils

    nc = _get_program(n_pairs=n_pairs, reps=reps, variant=variant)
    # pad inputs to the program's declared shapes (bias width varies by build)
    shapes = {}
    for alloc in nc.m.functions[0].allocations:
        if isinstance(alloc, mybir.MemoryLocationSet) and alloc.kind == "ExternalInput":
            shapes[alloc.memorylocations[0].name] = tuple(alloc.tensor_shape)
    fixed = []
    for m in in_maps:
        mm = {}
        for k, v in m.items():
            shp = shapes.get(k, tuple(v.shape))
            if tuple(v.shape) != shp:
                out = np.full(shp, SHIFT if k == "bias" else 0.0, v.dtype)
                sl = tuple(slice(0, min(s, t)) for s, t in zip(v.shape, shp))
                out[sl] = v[sl]
                mm[k] = out
            else:
                mm[k] = v
        fixed.append(mm)
    return bass_utils.run_bass_kernel_spmd(
        nc, fixed, core_ids=list(range(N_CORES)), trace=trace, **kwargs
    )


def kernel(text_feat, video_feat, text_mask):
    in_maps, perm, n_pairs = prepare_inputs(
        np.asarray(text_feat), np.asarray(video_feat), np.asarray(text_mask)
    )
    res = run(in_maps, n_pairs=n_pairs)
    packed = np.concatenate([res.results[c]["out"] for c in range(N_CORES)], axis=0)
    out = np.empty_like(packed)
    out[perm] = packed                      # un-permute queries to original order
    return out.astype(np.float32)


# revision 45
# speedup vs baseline: 1.1109x; 1.0045x over previous
"""Trainium2 Bass kernel for the two-level softmax-pooled text/video retrieval head.

Computes, for text_feat [256,32,512], video_feat [256,16,512], text_mask [256,32]:
    out[a,b] = (t2v(a,b) + v2t(a,b)) / 2
where t2v/v2t are two-level softmax-weighted poolings of the cross token/frame
cosine similarity tensor logits[a,b,t,v] (see reference module).

Sharding: text axis A split across 8 NeuronCores (32 queries each); video
features replicated. Host does l2-normalization + transposition (layout prep);
the device does all einsum + softmax compute.

v8 design: on top of the v7 pipeline (see below), VALID-TOKEN COMPACTION.
Lengths are in [16,32) (mean ~23.4), so ~27% of the (q,t) rows are padding
that v7 computed and then masked away via the 0/1 selector values.  v8
instead packs only the valid tokens of each query into consecutive rows and
lets the (already data-dependent) selector matmuls carry the arbitrary
row->query map.  Queries are assigned to cores by greedy LPT balancing so
every core's row count fits the same SPMD program (749 rows max vs the 768
capacity of 3 m-pairs for the reference inputs; a 4-pair program is compiled
as fallback if some other input needs it).  This cuts the main loop from 32
to 24 (pair,n) tiles: einsum, exp/copy, XE, v-tree and selector work all
drop ~25% across PE/ACT/DVE.  Mask values are baked into the text features
(t_row *= mask) so non-binary masks would still be exact.

v7 pipeline (per core, N_PAIR=3: 768 (q,t) rows; B=256, V=16 -> 4096 (b,v)
cols; D=512):
  - inputs are bf16 (0.4% feature rounding, ~1.7% first-level softmax weight
    noise -- well inside the 2e-2 tolerance); halves input DMA.
  - m-tiles are processed in PAIRS: one [128,1024] PSUM tile (2 banks) per
    pair, so ACT exp / ACT X-copy / DVE XE-mult each run at [128,1024] width.
  - E and XE are bf16; the XE multiply runs in the DVE 2x_1p mode.
  - sum-over-v (t2v level 1) is a 4-step strided add-tree on DVE (bf16 2x).
  - sum-over-t (v2t level 1 den/num) stays on PE as selector matmuls (bf16,
    exact 0/1, 32-wide stationary -> region-scoped PSUM accumulation),
    software-pipelined behind the main matmuls.
  - loop order is pair-outer / n-tile-inner within each b-half, so the t2v
    second level for a pair runs as soon as its half's columns are done.
"""

import sys

import numpy as np

if "/opt/trn_rl_repo" not in sys.path:
    sys.path.insert(0, "/opt/trn_rl_repo")

A, T_TOK, B, V_FRM, D = 256, 32, 256, 16, 512
N_CORES = 8
A_LOC = A // N_CORES            # 32 queries per core
N_COLS = B * V_FRM              # 4096  (b,v) cols
N_KC = D // 128                 # 4 K-chunks
N_NT = N_COLS // 512            # 8 N-tiles (32 videos each)
TAU = 100.0
SHIFT = -30.0                   # global exp shift (softmax-invariant)
EPS = 1e-6

_PROGRAM_CACHE = {}

# build-time scheduling knobs (A/B tuning; defaults = shipped config)
_TUNE = {"flush2": False, "dma_new": False, "pre_pop": False, "tail_order": 0,
         "e_bufs": 13, "skip": ""}


def _build_program(n_pairs=3, reps=1, variant=0):
    import contextlib

    import concourse.mybir as mybir
    import concourse.tile as tile
    from concourse import bacc

    N_PAIR = n_pairs
    N_MT = 2 * N_PAIR               # m-tiles of 128 rows
    M_ROWS = 128 * N_MT             # compacted (q,t) rows incl. padding

    f32 = mybir.dt.float32
    bf16 = mybir.dt.bfloat16
    EXP = mybir.ActivationFunctionType.Exp
    CPY = mybir.ActivationFunctionType.Copy
    MUL = mybir.AluOpType.mult
    ADD = mybir.AluOpType.add
    DIV = mybir.AluOpType.divide
    AX = mybir.AxisListType.X
    # The final /2 is folded into XE at creation (X-copy scale 0.5), so every
    # numerator downstream is pre-halved and each num/den ratio site becomes a
    # single DVE divide; the softmax exps compensate with scale 2*TAU.
    TAU2 = 2.0 * TAU

    nc = bacc.Bacc("TRN2", target_bir_lowering=False, debug=False)

    tT_d = nc.dram_tensor("tT", [D, M_ROWS], bf16, kind="ExternalInput")
    vT_d = nc.dram_tensor("vT", [D, N_COLS], bf16, kind="ExternalInput")
    sel_d = nc.dram_tensor("sel", [128, N_MT * 32], bf16, kind="ExternalInput")
    # bias width varies with reps/variant so each build gets a distinct HLO
    # hash (the NEFF cache otherwise silently reuses the first-compiled
    # program)
    bias_cols = 1 + (reps - 1) + 7 * variant
    bias_d = nc.dram_tensor("bias", [128, bias_cols], f32, kind="ExternalInput")
    out_d = nc.dram_tensor("out", [A_LOC, B], f32, kind="ExternalOutput")

    with tile.TileContext(nc) as tc, contextlib.ExitStack() as ctx:
        persist = ctx.enter_context(tc.tile_pool(name="persist", bufs=1))
        ps_pool = ctx.enter_context(tc.tile_pool(name="ps", bufs=2, space="PSUM"))
        dn2_pool = ctx.enter_context(tc.tile_pool(name="dn2", bufs=1, space="PSUM"))
        dn3_pool = ctx.enter_context(tc.tile_pool(name="dn3", bufs=1, space="PSUM"))
        e_pool = ctx.enter_context(tc.tile_pool(name="e", bufs=_TUNE["e_bufs"]))
        tr_pool = ctx.enter_context(tc.tile_pool(name="tr", bufs=3))
        xb_pool = ctx.enter_context(tc.tile_pool(name="xb", bufs=3))
        t2v_pool = ctx.enter_context(tc.tile_pool(name="t2v", bufs=2))
        w_pool = ctx.enter_context(tc.tile_pool(name="w", bufs=3))
        v_pool = ctx.enter_context(tc.tile_pool(name="v2", bufs=2))

        # ---- persistent inputs. Four issue queues in parallel (SP/DVE/ACT
        # HWDGE + Pool SWDGE) so descriptor generation (~0.6-1us per DMA) is
        # not the startup bottleneck.  Critical-path tiles (first pair's text
        # chunks, the n=0/1 video tiles) go out as small single transfers in
        # consumption order; everything later is batched into wide transfers.
        tt_tiles = []
        vt_tiles = {}
        for k in range(N_KC):
            t_ = persist.tile([128, M_ROWS], bf16, tag=f"tt_{k}")
            tt_tiles.append(t_)

        def vt_chunk(k, n0, n1, q):
            t_ = persist.tile([128, 512 * (n1 - n0)], bf16, tag=f"vt_{k}_{n0}")
            q.dma_start(out=t_[:],
                        in_=vT_d.ap()[128 * k:128 * (k + 1),
                                      512 * n0:512 * n1])
            for n in range(n0, n1):
                vt_tiles[(k, n)] = t_[:, 512 * (n - n0):512 * (n - n0 + 1)]

        if _TUNE["dma_new"]:
            # text heads (first pair) + first video tiles: smallest possible
            # critical path to the first matmul.  HWDGE queues: SP carries the
            # text, ACT carries the first-half video; the Pool SWDGE queue
            # takes the constants + late n-tiles.
            tt_head = min(256, M_ROWS)
            for k in range(N_KC):
                nc.sync.dma_start(out=tt_tiles[k][:, 0:tt_head],
                                  in_=tT_d.ap()[128 * k:128 * (k + 1),
                                                0:tt_head])
                vt_chunk(k, 0, 1, nc.scalar)
            vt_chunk(2, 1, 2, nc.gpsimd)
            vt_chunk(3, 1, 2, nc.gpsimd)
            bias_sb = persist.tile([128, bias_cols], f32, tag="bias")
            nc.gpsimd.dma_start(out=bias_sb[:], in_=bias_d.ap())
            sel_sb = persist.tile([128, N_MT * 32], bf16, tag="sel")
            nc.gpsimd.dma_start(out=sel_sb[:], in_=sel_d.ap())
            vt_chunk(0, 1, 2, nc.scalar)
            vt_chunk(1, 1, 2, nc.scalar)
            # warm the ACT spline tables (exp) while the first einsum runs:
            # the auto-inserted table load lands before this dep-free dummy
            warm = persist.tile([128, 1], f32, tag="warm")
            nc.scalar.activation(warm[:], nc.const_aps.tensor(0.0, (128, 1)),
                                 EXP, bias=0.0, scale=1.0)
            if M_ROWS > tt_head:
                for k in range(N_KC):
                    nc.sync.dma_start(
                        out=tt_tiles[k][:, tt_head:M_ROWS],
                        in_=tT_d.ap()[128 * k:128 * (k + 1), tt_head:M_ROWS])
            vt_chunk(0, 2, 4, nc.scalar)
            vt_chunk(1, 2, 4, nc.scalar)
            vt_chunk(2, 2, 4, nc.gpsimd)
            vt_chunk(3, 2, 4, nc.gpsimd)
            for k in range(N_KC):
                vt_chunk(k, 4, N_NT, nc.gpsimd)
        else:
            # v7/v8.0 DMA order: SP/Pool alternation, per-n single transfers.
            # The ACT HWDGE queue carries two of the first-tile video loads
            # so the Pool queue reaches the n=1 tiles ~2us earlier.
            _dma_q = [nc.sync, nc.gpsimd]
            use_sc = _TUNE.get("scalar_singles", 0)
            tt_head = min(_TUNE.get("tt_head", 512), M_ROWS)
            for k in range(N_KC):
                _dma_q[0].dma_start(out=tt_tiles[k][:, 0:tt_head],
                                    in_=tT_d.ap()[128 * k:128 * (k + 1),
                                                  0:tt_head])
                vt_chunk(k, 0, 1,
                         nc.scalar if k >= N_KC - use_sc else _dma_q[1])
            plan = _TUNE.get("dma_plan", 1)
            # n=1 video: the k=1,3 tiles ride the ACT HWDGE queue (plan>=2)
            # so they don't queue behind the four n=0 SWDGE issues
            n1q = {0: _dma_q, 1: _dma_q,
                   2: [nc.sync, nc.scalar]}[min(plan, 2)]
            for k in range(N_KC):
                vt_chunk(k, 1, 2, n1q[k % 2])
            # constants ride the otherwise-idle ACT HWDGE queue so the n=1
            # video tiles aren't queued behind them on SP
            bias_sb = persist.tile([128, bias_cols], f32, tag="bias")
            sel_sb = persist.tile([128, N_MT * 32], bf16, tag="sel")
            cq = nc.scalar if _TUNE.get("const_scalar", True) else nc.sync
            cq.dma_start(out=bias_sb[:], in_=bias_d.ap())
            cq.dma_start(out=sel_sb[:], in_=sel_d.ap())
            if _TUNE.get("warm_act", True):
                # warm the ACT spline tables while the first einsum runs
                warm = persist.tile([128, 1], f32, tag="warm")
                nc.scalar.activation(warm[:],
                                     nc.const_aps.tensor(0.0, (128, 1)),
                                     EXP, bias=0.0, scale=1.0)
            if plan >= 3:
                for k in range(N_KC):
                    vt_chunk(k, 2, 4, _dma_q[k % 2])
            else:
                for k in range(N_KC):
                    vt_chunk(k, 2, 3, _dma_q[k % 2])
                for k in range(N_KC):
                    vt_chunk(k, 3, 4, _dma_q[k % 2])
            if M_ROWS > tt_head:
                for k in range(N_KC):
                    _dma_q[k % 2].dma_start(
                        out=tt_tiles[k][:, tt_head:M_ROWS],
                        in_=tT_d.ap()[128 * k:128 * (k + 1), tt_head:M_ROWS])
            for n in range(4, N_NT):
                for k in range(N_KC):
                    vt_chunk(k, n, n + 1, _dma_q[k % 2])

        # S|N accumulator, side-major: col = side*(256*N_MT) + m*256 + n*32 + b
        sn_all = persist.tile([128, 2 * N_MT * 256], f32, tag="sn_all")
        # final staging, (j,q)-partition layout: row = 32j+q, col = 32h+b'
        # (video index = 128h + 32j + b'); avoids any partition-crossing DMA
        # until the single strided output DMA
        vt2_full = persist.tile([128, 64], f32, tag="vt2_full")
        bias0 = bias_sb[:, 0:1]

        def issue_sel(den2, num2, item):
            """The 4 over-t selector matmuls for one queued pair-tile.
            32-wide stationary: band j of den2/num2 is its own region-scoped
            accumulation group, complete at (mp==N_PAIR-1, mi==1)."""
            j, mp, exe = item
            for mi in range(2):
                m = 2 * mp + mi
                selm = sel_sb[:, 32 * m:32 * (m + 1)]
                band = slice(32 * j, 32 * (j + 1))
                nc.tensor.matmul(
                    den2[band, :], selm, exe[:, 512 * mi:512 * (mi + 1)],
                    start=(mp == 0 and mi == 0), stop=(mp == N_PAIR - 1 and mi == 1),
                    skip_group_check=True, tile_position=(0, 32 * j),
                )
                nc.tensor.matmul(
                    num2[band, :], selm, exe[:, 1024 + 512 * mi:1536 + 512 * mi],
                    start=(mp == 0 and mi == 0), stop=(mp == N_PAIR - 1 and mi == 1),
                    skip_group_check=True, tile_position=(0, 32 * j),
                )

        for _rep in range(reps):
            # t2v second-level accumulators in the (j,q)/(h,b') layout:
            # row = 32j+q, col = 32h+b'
            den3 = dn3_pool.tile([128, 64], f32, tag="den3")
            num3 = dn3_pool.tile([128, 64], f32, tag="num3")

            # --- phase-2 (t2v second level) for one m-tile and one b-half;
            # interleaved into the main loop right where its inputs are
            # ready.  Split into an a-stage (DVE ratio + ACT exp) and a
            # b-stage (DVE weight product + PE matmuls) so tail emission can
            # interleave several chains without idling DVE on ACT hops.
            def t2v_level2_a(mp, h):
                """t2v second level for BOTH m-tiles of pair mp at [128,256]
                width (their sn_all columns complete simultaneously)."""
                hb = 128 * h
                sn4 = sn_all[:].rearrange("p (s m c) -> p s m c", s=2, m=N_MT)
                s1 = sn4[:, 0, 2 * mp:2 * mp + 2, hb:hb + 128]
                n1 = sn4[:, 1, 2 * mp:2 * mp + 2, hb:hb + 128]
                rs1 = t2v_pool.tile([128, 256], f32, tag="rs1")
                nc.vector.reciprocal(rs1[:], s1)
                t2v_t = t2v_pool.tile([128, 256], f32, tag="t2v_t")
                nc.vector.tensor_tensor(t2v_t[:], n1, rs1[:], op=MUL)
                w_t = w_pool.tile([128, 256], bf16, tag="w_t")
                nc.scalar.activation(w_t[:], t2v_t[:], EXP, bias=bias0,
                                     scale=TAU2)
                return t2v_t, w_t

            def t2v_level2_b(mp, h, t2v_t, w_t):
                xw_t = w_pool.tile([128, 256], bf16, tag="xw_t")
                with nc.allow_low_precision(reason="bf16 softmax weights"):
                    nc.vector.tensor_tensor(xw_t[:], t2v_t[:], w_t[:], op=MUL)
                hcol = slice(32 * h, 32 * h + 32)
                for mi in range(2):
                    m = 2 * mp + mi
                    selm = sel_sb[:, 32 * m:32 * (m + 1)]
                    for j in range(4):
                        band = slice(32 * j, 32 * (j + 1))
                        wcol = slice(128 * mi + 32 * j, 128 * mi + 32 * j + 32)
                        nc.tensor.matmul(
                            den3[band, hcol], selm, w_t[:, wcol],
                            start=(m == 0), stop=(m == N_MT - 1),
                            skip_group_check=True, tile_position=(0, 32 * j),
                        )
                        nc.tensor.matmul(
                            num3[band, hcol], selm, xw_t[:, wcol],
                            start=(m == 0), stop=(m == N_MT - 1),
                            skip_group_check=True, tile_position=(0, 32 * j),
                        )

            def t2v_level2(mp, h):
                t2v_t, w_t = t2v_level2_a(mp, h)
                t2v_level2_b(mp, h, t2v_t, w_t)

            # ---- main loop: halves (b 0:128 / 128:256) x M-pairs x N-tiles
            for h in range(2):
                den2 = dn2_pool.tile([128, 512], f32, tag="den2")
                num2 = dn2_pool.tile([128, 512], f32, tag="num2")
                sel_queue = []   # software pipeline: sel MMs run 2 tiles late
                for mp in range(N_PAIR):
                    for j in range(4):
                        n = 4 * h + j
                        # drain queued selector matmuls BEFORE this tile's
                        # einsum: their deps are >=1 tile old (>=4 normally),
                        # so they slot into the PE stream without waiting,
                        # and the last pair drains 2/tile so nearly nothing
                        # serializes after the final XE.
                        if _TUNE["pre_pop"]:
                            npop = 2 if mp == N_PAIR - 1 else (
                                1 if len(sel_queue) > 4 else 0)
                            for _ in range(npop):
                                if sel_queue:
                                    issue_sel(den2, num2, sel_queue.pop(0))
                        ps = ps_pool.tile([128, 1024], f32, tag="ps")
                        for mi in range(2):
                            m = 2 * mp + mi
                            for k in range(N_KC):
                                nc.tensor.matmul(
                                    ps[:, 512 * mi:512 * (mi + 1)],
                                    tt_tiles[k][:, 128 * m:128 * (m + 1)],
                                    vt_tiles[(k, n)],
                                    start=(k == 0),
                                    stop=(k == N_KC - 1),
                                )
                        # E|XE for the pair: [E(m0) E(m1) XE(m0) XE(m1)]
                        exe = e_pool.tile([128, 2048], bf16, tag="exe")
                        # the last pair of the half gates the v2t second
                        # level: split exp/mult per m-tile and multiply
                        # straight from PSUM (1x mode) so its selector
                        # matmuls issue ~1us earlier
                        tail_pair = (mp == N_PAIR - 1 and j == 3)
                        with nc.allow_low_precision(reason="bf16 E/XE"):
                            if tail_pair:
                                # both exps emitted adjacently: keeps the
                                # scheduler's coalesced ACT->PE wait from
                                # covering the selector matmuls in between
                                for mi in range(2):
                                    ms = slice(512 * mi, 512 * mi + 512)
                                    nc.scalar.activation(
                                        exe[:, ms], ps[:, ms], EXP,
                                        bias=bias0, scale=TAU)
                                for mi in range(2):
                                    ms = slice(512 * mi, 512 * mi + 512)
                                    xs = slice(1024 + 512 * mi,
                                               1536 + 512 * mi)
                                    nc.vector.scalar_tensor_tensor(
                                        out=exe[:, xs], in0=ps[:, ms],
                                        scalar=0.5, in1=exe[:, ms],
                                        op0=MUL, op1=MUL)
                            else:
                                nc.scalar.activation(
                                    exe[:, 0:1024], ps[:], EXP, bias=bias0,
                                    scale=TAU)
                                xbt = xb_pool.tile([128, 1024], bf16,
                                                   tag="xbt")
                                # the X-copy carries the global 0.5 fold
                                nc.scalar.activation(xbt[:], ps[:], CPY,
                                                     bias=0.0, scale=0.5)
                                nc.vector.tensor_tensor(
                                    exe[:, 1024:2048], xbt[:], exe[:, 0:1024],
                                    op=MUL)
                        # queue the 4 selector matmuls (over-t sums on PE),
                        # issued a few tiles later so their ACT/DVE deps are
                        # done by the time PE reaches them.  In the last pair
                        # the queue drains 2/tile so den2/num2 close right
                        # after the final XE instead of 4 items later.
                        sel_queue.append((j, mp, exe))
                        lag = _TUNE.get("lag0", 4) if (h == 0 and mp == 0) \
                            else 4
                        if len(sel_queue) > lag or tail_pair:
                            issue_sel(den2, num2, sel_queue.pop(0))
                        # sum over v: 4-step strided bf16 add-tree on DVE.
                        # tr: [0:1024]=(c4,b32,v8)  [1024:1536]=(c,b,4)
                        #     [1536:1792]=(c,b,2)
                        tr = tr_pool.tile([128, 1792], bf16, tag="tr")
                        ev16 = exe[:].rearrange("p (c b v) -> p c b v", c=4,
                                                v=16)
                        t8 = tr[:, 0:1024].rearrange("p (c b v) -> p c b v",
                                                     c=4, v=8)
                        t4 = tr[:, 1024:1536].rearrange("p (c b v) -> p c b v",
                                                        c=4, v=4)
                        t2 = tr[:, 1536:1792].rearrange("p (c b v) -> p c b v",
                                                        c=4, v=2)
                        with nc.allow_low_precision(reason="bf16 v-tree"):
                            nc.vector.tensor_tensor(
                                t8[:], ev16[:, :, :, 0:8], ev16[:, :, :, 8:16],
                                op=ADD)
                            nc.vector.tensor_tensor(
                                t4[:], t8[:, :, :, 0:4], t8[:, :, :, 4:8],
                                op=ADD)
                            nc.vector.tensor_tensor(
                                t2[:], t4[:, :, :, 0:2], t4[:, :, :, 2:4],
                                op=ADD)
                        # final add writes fp32 S|N: col = s*(256*N_MT)+m*256+n*32+b
                        snv = sn_all[:].rearrange(
                            "p (s m nb) -> p s m nb", s=2, m=N_MT)[
                            :, :, 2 * mp:2 * mp + 2, 32 * n:32 * (n + 1)]
                        nc.vector.tensor_tensor(
                            snv, t2[:, :, :, 0:1], t2[:, :, :, 1:2], op=ADD)
                    # this pair's half-columns are complete: run its t2v
                    # second level overlapped with the remaining pairs (the
                    # last pair's runs after the v2t chain below, which is
                    # the longer critical path at the end of the half)
                    if mp < N_PAIR - 1:
                        t2v_level2(mp, h)
                while sel_queue:
                    issue_sel(den2, num2, sel_queue.pop(0))

                # ---- second level of v2t for this half (softmax over v),
                # split into two column groups (b' 0:16 / 16:32) so the serial
                # chain pipelines and the last-half tail is ~2x shorter ----
                v_t = v_pool.tile([128, 512], f32, tag="v_t")
                exev = v_pool.tile([128, 1024], bf16, tag="exev")
                snv_t = v_pool.tile([128, 64], f32, tag="snv_t")
                ev16 = exev[:].rearrange("p (c b v) -> p c b v", c=2, v=16)
                sn8 = snv_t[:].rearrange("p (c b) -> p c b", c=2)
                # ACT (idle here) can evacuate den2/num2 from PSUM so the DVE
                # reciprocal+multiply read SBUF (no PSUM access penalty)
                if _TUNE.get("evac", False):
                    den2c = v_pool.tile([128, 512], f32, tag="den2c")
                    num2c = v_pool.tile([128, 512], f32, tag="num2c")
                    nc.scalar.activation(den2c[:], den2[:], CPY)
                    nc.scalar.activation(num2c[:], num2[:], CPY)
                else:
                    den2c, num2c = den2, num2

                rden2 = v_pool.tile([128, 512], f32, tag="rden2")

                def v2t_a(cg):
                    cs = slice(256 * cg, 256 * cg + 256)
                    nc.vector.reciprocal(rden2[:, cs], den2c[:, cs])
                    nc.vector.tensor_tensor(
                        v_t[:, cs], num2c[:, cs], rden2[:, cs], op=MUL)
                    nc.scalar.activation(
                        exev[:, cs], v_t[:, cs], EXP, bias=bias0, scale=TAU2)

                def v2t_b(cg):
                    cs = slice(256 * cg, 256 * cg + 256)
                    bs = slice(16 * cg, 16 * cg + 16)
                    with nc.allow_low_precision(reason="bf16 Ev/XEv"):
                        nc.vector.tensor_tensor(
                            exev[:, 512 + 256 * cg:768 + 256 * cg],
                            v_t[:, cs], exev[:, cs], op=MUL)
                    # single reduce (no fast modes either way) instead of an
                    # add-tree: 1 instruction of latency on the tail chain.
                    # Optionally on the (idle) Pool engine to relieve DVE,
                    # which is the throughput bound of this tail.
                    red_eng = nc.gpsimd if _TUNE.get("pool_red", False) \
                        else nc.vector
                    red_eng.reduce_sum(
                        out=sn8[:, :, bs],
                        in_=ev16[:, :, bs, :],
                        axis=AX)
                    # vt2 = Nv'/Sv (the final /2 arrived via the XE fold),
                    # written straight into the (j,q)/(h,b') staging tile
                    rsv_t = v_pool.tile([128, 32], f32, tag="rsv_t")
                    nc.vector.reciprocal(rsv_t[:, bs], snv_t[:, bs])
                    nc.vector.tensor_tensor(
                        vt2_full[:, 32 * h + 16 * cg:32 * h + 16 * cg + 16],
                        snv_t[:, 32:64][:, bs], rsv_t[:, bs], op=MUL)

                # interleave the v2t chains, the deferred last pair's t2v
                # level 2, and phase 3 so DVE always has ready work while
                # ACT exps round-trip
                mp_last = N_PAIR - 1
                v2t_a(0)
                ta = t2v_level2_a(mp_last, h)
                v2t_a(1)
                v2t_b(0)
                t2v_level2_b(mp_last, h, *ta)
                v2t_b(1)

                # ---- phase 3 for this half: t2v2 = 0.5*Num3/Den3, combine
                # with v2t, and DMA out[q, 128h+32j+b'].  den3/num3's h-region
                # groups stop at m==N_MT-1 just above, so the h=0 finale fully
                # overlaps the h=1 main loop. ----
                hcol = slice(32 * h, 32 * h + 32)
                rden3 = t2v_pool.tile([128, 32], f32, tag="rden3")
                nc.vector.reciprocal(rden3[:], den3[:, hcol])
                t2v2 = t2v_pool.tile([128, 32], f32, tag="t2v2")
                nc.vector.tensor_tensor(t2v2[:], num3[:, hcol], rden3[:],
                                        op=MUL)
                out_sb = t2v_pool.tile([128, 32], f32, tag="out_sb")
                nc.vector.tensor_tensor(out_sb[:], t2v2[:],
                                        vt2_full[:, hcol], op=ADD)
                out_ap = out_d.ap().rearrange(
                    "q (h j b) -> h j q b", h=2, j=4)[h:h + 1]
                nc.sync.dma_start(out=out_ap, in_=out_sb[:])

    nc.compile()
    return nc


def _get_program(n_pairs=3, reps=1, variant=0, **_ignored):
    key = (n_pairs, reps, variant)
    if key not in _PROGRAM_CACHE:
        _PROGRAM_CACHE[key] = _build_program(n_pairs, reps, variant)
    return _PROGRAM_CACHE[key]


def _l2norm(a):
    n = np.linalg.norm(a, axis=-1, keepdims=True)
    return a / np.maximum(n, EPS)


def _bf16(a):
    import ml_dtypes
    return a.astype(ml_dtypes.bfloat16)


def _assign_queries(lengths):
    """Greedy LPT: partition the A queries into N_CORES groups of A_LOC,
    minimizing the max total token count per core."""
    order = np.argsort(-lengths, kind="stable")
    core_sum = np.zeros(N_CORES, np.int64)
    core_cnt = np.zeros(N_CORES, np.int64)
    groups = [[] for _ in range(N_CORES)]
    for q in order:
        open_c = np.nonzero(core_cnt < A_LOC)[0]
        c = open_c[np.argmin(core_sum[open_c])]
        groups[c].append(int(q))
        core_sum[c] += lengths[q]
        core_cnt[c] += 1
    return groups, int(core_sum.max())


def prepare_inputs(text_feat, video_feat, text_mask):
    """Host-side shard/layout prep with valid-token compaction.
    Returns (in_maps for the 8 cores, query permutation, n_pairs)."""
    t = _l2norm(text_feat.astype(np.float32))          # [A, T, D]
    v = _l2norm(video_feat.astype(np.float32))         # [B, V, D]
    mask = text_mask.astype(np.float32)

    # bake mask values into the text rows: logits = (mask*t) . v exactly
    tm = t * mask[:, :, None]

    lengths = (mask != 0.0).sum(axis=1).astype(np.int64)   # valid tokens per query
    groups, max_rows = _assign_queries(lengths)
    n_pairs = max(1, -(-max_rows // 256))                   # ceil to 256-row pairs
    m_rows = 256 * n_pairs
    n_mt = 2 * n_pairs

    # video: [B, V, D] -> [D, B*V], shared by all cores
    vT = _bf16(np.ascontiguousarray(v.reshape(B * V_FRM, D).T))

    in_maps = []
    perm = np.empty(A, np.int64)
    for c in range(N_CORES):
        qs = groups[c]
        perm[c * A_LOC:(c + 1) * A_LOC] = qs
        rows = np.zeros((m_rows, D), np.float32)
        sel = np.zeros((128, n_mt * 32), np.float32)
        r = 0
        for jloc, q in enumerate(qs):
            idx = np.nonzero(mask[q] != 0.0)[0]
            k = len(idx)
            rows[r:r + k] = tm[q, idx]
            rr = np.arange(r, r + k)
            sel[rr % 128, (rr // 128) * 32 + jloc] = 1.0
            r += k
        tT = _bf16(np.ascontiguousarray(rows.T))            # [D, m_rows]
        bias = np.full((128, 1), SHIFT, np.float32)
        in_maps.append({"tT": tT, "vT": vT, "sel": _bf16(sel), "bias": bias})
    return in_maps, perm, n_pairs


def run(in_maps, n_pairs=3, trace=False, reps=1, variant=0, **kwargs):
    import concourse.mybir as mybir
    from concourse import bass_utils

    nc = _get_program(n_pairs=n_pairs, reps=reps, variant=variant)
    # pad inputs to the program's declared shapes (bias width varies by build)
    shapes = {}
    for alloc in nc.m.functions[0].allocations:
        if isinstance(alloc, mybir.MemoryLocationSet) and alloc.kind == "ExternalInput":
            shapes[alloc.memorylocations[0].name] = tuple(alloc.tensor_shape)
    fixed = []
    for m in in_maps:
        mm = {}
        for k, v in m.items():
            shp = shapes.get(k, tuple(v.shape))
            if tuple(v.shape) != shp:
                out = np.full(shp, SHIFT if k == "bias" else 0.0, v.dtype)
                sl = tuple(slice(0, min(s, t)) for s, t in zip(v.shape, shp))
                out[sl] = v[sl]
                mm[k] = out
            else:
                mm[k] = v
        fixed.append(mm)
    return bass_utils.run_bass_kernel_spmd(
        nc, fixed, core_ids=list(range(N_CORES)), trace=trace, **kwargs
    )


def kernel(text_feat, video_feat, text_mask):
    in_maps, perm, n_pairs = prepare_inputs(
        np.asarray(text_feat), np.asarray(video_feat), np.asarray(text_mask)
    )
    res = run(in_maps, n_pairs=n_pairs)
    packed = np.concatenate([res.results[c]["out"] for c in range(N_CORES)], axis=0)
    out = np.empty_like(packed)
    out[perm] = packed                      # un-permute queries to original order
    return out.astype(np.float32)


# revision 46
# speedup vs baseline: 1.1197x; 1.0079x over previous
"""Trainium2 Bass kernel for the two-level softmax-pooled text/video retrieval head.

Computes, for text_feat [256,32,512], video_feat [256,16,512], text_mask [256,32]:
    out[a,b] = (t2v(a,b) + v2t(a,b)) / 2
where t2v/v2t are two-level softmax-weighted poolings of the cross token/frame
cosine similarity tensor logits[a,b,t,v] (see reference module).

Sharding: text axis A split across 8 NeuronCores (32 queries each); video
features replicated. Host does l2-normalization + transposition (layout prep);
the device does all einsum + softmax compute.

v8 design: on top of the v7 pipeline (see below), VALID-TOKEN COMPACTION.
Lengths are in [16,32) (mean ~23.4), so ~27% of the (q,t) rows are padding
that v7 computed and then masked away via the 0/1 selector values.  v8
instead packs only the valid tokens of each query into consecutive rows and
lets the (already data-dependent) selector matmuls carry the arbitrary
row->query map.  Queries are assigned to cores by greedy LPT balancing so
every core's row count fits the same SPMD program (749 rows max vs the 768
capacity of 3 m-pairs for the reference inputs; a 4-pair program is compiled
as fallback if some other input needs it).  This cuts the main loop from 32
to 24 (pair,n) tiles: einsum, exp/copy, XE, v-tree and selector work all
drop ~25% across PE/ACT/DVE.  Mask values are baked into the text features
(t_row *= mask) so non-binary masks would still be exact.

v7 pipeline (per core, N_PAIR=3: 768 (q,t) rows; B=256, V=16 -> 4096 (b,v)
cols; D=512):
  - inputs are bf16 (0.4% feature rounding, ~1.7% first-level softmax weight
    noise -- well inside the 2e-2 tolerance); halves input DMA.
  - m-tiles are processed in PAIRS: one [128,1024] PSUM tile (2 banks) per
    pair, so ACT exp / ACT X-copy / DVE XE-mult each run at [128,1024] width.
  - E and XE are bf16; the XE multiply runs in the DVE 2x_1p mode.
  - sum-over-v (t2v level 1) is a 4-step strided add-tree on DVE (bf16 2x).
  - sum-over-t (v2t level 1 den/num) stays on PE as selector matmuls (bf16,
    exact 0/1, 32-wide stationary -> region-scoped PSUM accumulation),
    software-pipelined behind the main matmuls.
  - loop order is pair-outer / n-tile-inner within each b-half, so the t2v
    second level for a pair runs as soon as its half's columns are done.
"""

import sys

import numpy as np

if "/opt/trn_rl_repo" not in sys.path:
    sys.path.insert(0, "/opt/trn_rl_repo")

A, T_TOK, B, V_FRM, D = 256, 32, 256, 16, 512
N_CORES = 8
A_LOC = A // N_CORES            # 32 queries per core
N_COLS = B * V_FRM              # 4096  (b,v) cols
N_KC = D // 128                 # 4 K-chunks
N_NT = N_COLS // 512            # 8 N-tiles (32 videos each)
TAU = 100.0
SHIFT = -30.0                   # global exp shift (softmax-invariant)
EPS = 1e-6

_PROGRAM_CACHE = {}

# build-time scheduling knobs (A/B tuning; defaults = shipped config)
_TUNE = {"flush2": False, "dma_new": False, "pre_pop": False, "tail_order": 0,
         "e_bufs": 13, "skip": ""}


def _build_program(n_pairs=3, reps=1, variant=0):
    import contextlib

    import concourse.mybir as mybir
    import concourse.tile as tile
    from concourse import bacc

    N_PAIR = n_pairs
    N_MT = 2 * N_PAIR               # m-tiles of 128 rows
    M_ROWS = 128 * N_MT             # compacted (q,t) rows incl. padding

    f32 = mybir.dt.float32
    bf16 = mybir.dt.bfloat16
    EXP = mybir.ActivationFunctionType.Exp
    CPY = mybir.ActivationFunctionType.Copy
    MUL = mybir.AluOpType.mult
    ADD = mybir.AluOpType.add
    DIV = mybir.AluOpType.divide
    AX = mybir.AxisListType.X
    # The final /2 is folded into XE at creation (X-copy scale 0.5), so every
    # numerator downstream is pre-halved and each num/den ratio site becomes a
    # single DVE divide; the softmax exps compensate with scale 2*TAU.
    TAU2 = 2.0 * TAU

    nc = bacc.Bacc("TRN2", target_bir_lowering=False, debug=False)

    tT_d = nc.dram_tensor("tT", [D, M_ROWS], bf16, kind="ExternalInput")
    vT_d = nc.dram_tensor("vT", [D, N_COLS], bf16, kind="ExternalInput")
    sel_d = nc.dram_tensor("sel", [128, N_MT * 32], bf16, kind="ExternalInput")
    # bias width varies with reps/variant so each build gets a distinct HLO
    # hash (the NEFF cache otherwise silently reuses the first-compiled
    # program)
    bias_cols = 1 + (reps - 1) + 7 * variant
    bias_d = nc.dram_tensor("bias", [128, bias_cols], f32, kind="ExternalInput")
    out_d = nc.dram_tensor("out", [A_LOC, B], f32, kind="ExternalOutput")

    with tile.TileContext(nc) as tc, contextlib.ExitStack() as ctx:
        persist = ctx.enter_context(tc.tile_pool(name="persist", bufs=1))
        ps_pool = ctx.enter_context(tc.tile_pool(name="ps", bufs=2, space="PSUM"))
        dn2_pool = ctx.enter_context(tc.tile_pool(name="dn2", bufs=1, space="PSUM"))
        dn3_pool = ctx.enter_context(tc.tile_pool(name="dn3", bufs=1, space="PSUM"))
        e_pool = ctx.enter_context(tc.tile_pool(name="e", bufs=_TUNE["e_bufs"]))
        tr_pool = ctx.enter_context(tc.tile_pool(name="tr", bufs=3))
        xb_pool = ctx.enter_context(tc.tile_pool(name="xb", bufs=3))
        t2v_pool = ctx.enter_context(tc.tile_pool(name="t2v", bufs=2))
        w_pool = ctx.enter_context(tc.tile_pool(name="w", bufs=3))
        v_pool = ctx.enter_context(tc.tile_pool(name="v2", bufs=2))

        # ---- persistent inputs. Four issue queues in parallel (SP/DVE/ACT
        # HWDGE + Pool SWDGE) so descriptor generation (~0.6-1us per DMA) is
        # not the startup bottleneck.  Critical-path tiles (first pair's text
        # chunks, the n=0/1 video tiles) go out as small single transfers in
        # consumption order; everything later is batched into wide transfers.
        tt_tiles = []
        vt_tiles = {}
        for k in range(N_KC):
            t_ = persist.tile([128, M_ROWS], bf16, tag=f"tt_{k}")
            tt_tiles.append(t_)

        def vt_chunk(k, n0, n1, q):
            t_ = persist.tile([128, 512 * (n1 - n0)], bf16, tag=f"vt_{k}_{n0}")
            q.dma_start(out=t_[:],
                        in_=vT_d.ap()[128 * k:128 * (k + 1),
                                      512 * n0:512 * n1])
            for n in range(n0, n1):
                vt_tiles[(k, n)] = t_[:, 512 * (n - n0):512 * (n - n0 + 1)]

        if _TUNE["dma_new"]:
            # text heads (first pair) + first video tiles: smallest possible
            # critical path to the first matmul.  HWDGE queues: SP carries the
            # text, ACT carries the first-half video; the Pool SWDGE queue
            # takes the constants + late n-tiles.
            tt_head = min(256, M_ROWS)
            for k in range(N_KC):
                nc.sync.dma_start(out=tt_tiles[k][:, 0:tt_head],
                                  in_=tT_d.ap()[128 * k:128 * (k + 1),
                                                0:tt_head])
                vt_chunk(k, 0, 1, nc.scalar)
            vt_chunk(2, 1, 2, nc.gpsimd)
            vt_chunk(3, 1, 2, nc.gpsimd)
            bias_sb = persist.tile([128, bias_cols], f32, tag="bias")
            nc.gpsimd.dma_start(out=bias_sb[:], in_=bias_d.ap())
            sel_sb = persist.tile([128, N_MT * 32], bf16, tag="sel")
            nc.gpsimd.dma_start(out=sel_sb[:], in_=sel_d.ap())
            vt_chunk(0, 1, 2, nc.scalar)
            vt_chunk(1, 1, 2, nc.scalar)
            # warm the ACT spline tables (exp) while the first einsum runs:
            # the auto-inserted table load lands before this dep-free dummy
            warm = persist.tile([128, 1], f32, tag="warm")
            nc.scalar.activation(warm[:], nc.const_aps.tensor(0.0, (128, 1)),
                                 EXP, bias=0.0, scale=1.0)
            if M_ROWS > tt_head:
                for k in range(N_KC):
                    nc.sync.dma_start(
                        out=tt_tiles[k][:, tt_head:M_ROWS],
                        in_=tT_d.ap()[128 * k:128 * (k + 1), tt_head:M_ROWS])
            vt_chunk(0, 2, 4, nc.scalar)
            vt_chunk(1, 2, 4, nc.scalar)
            vt_chunk(2, 2, 4, nc.gpsimd)
            vt_chunk(3, 2, 4, nc.gpsimd)
            for k in range(N_KC):
                vt_chunk(k, 4, N_NT, nc.gpsimd)
        else:
            # v7/v8.0 DMA order: SP/Pool alternation, per-n single transfers.
            # The ACT HWDGE queue carries two of the first-tile video loads
            # so the Pool queue reaches the n=1 tiles ~2us earlier.
            _dma_q = [nc.sync, nc.gpsimd]
            use_sc = _TUNE.get("scalar_singles", 0)
            tt_head = min(_TUNE.get("tt_head", 512), M_ROWS)
            for k in range(N_KC):
                _dma_q[0].dma_start(out=tt_tiles[k][:, 0:tt_head],
                                    in_=tT_d.ap()[128 * k:128 * (k + 1),
                                                  0:tt_head])
                vt_chunk(k, 0, 1,
                         nc.scalar if k >= N_KC - use_sc else _dma_q[1])
            plan = _TUNE.get("dma_plan", 1)
            # n=1 video: the k=1,3 tiles ride the ACT HWDGE queue (plan>=2)
            # so they don't queue behind the four n=0 SWDGE issues
            n1q = {0: _dma_q, 1: _dma_q,
                   2: [nc.sync, nc.scalar]}[min(plan, 2)]
            for k in range(N_KC):
                vt_chunk(k, 1, 2, n1q[k % 2])
            # constants ride the otherwise-idle ACT HWDGE queue so the n=1
            # video tiles aren't queued behind them on SP
            bias_sb = persist.tile([128, bias_cols], f32, tag="bias")
            sel_sb = persist.tile([128, N_MT * 32], bf16, tag="sel")
            cq = nc.scalar if _TUNE.get("const_scalar", True) else nc.sync
            cq.dma_start(out=bias_sb[:], in_=bias_d.ap())
            cq.dma_start(out=sel_sb[:], in_=sel_d.ap())
            if _TUNE.get("warm_act", True):
                # warm the ACT spline tables while the first einsum runs
                warm = persist.tile([128, 1], f32, tag="warm")
                nc.scalar.activation(warm[:],
                                     nc.const_aps.tensor(0.0, (128, 1)),
                                     EXP, bias=0.0, scale=1.0)
            if plan >= 3:
                for k in range(N_KC):
                    vt_chunk(k, 2, 4, _dma_q[k % 2])
            else:
                for k in range(N_KC):
                    vt_chunk(k, 2, 3, _dma_q[k % 2])
                for k in range(N_KC):
                    vt_chunk(k, 3, 4, _dma_q[k % 2])
            if M_ROWS > tt_head:
                for k in range(N_KC):
                    _dma_q[k % 2].dma_start(
                        out=tt_tiles[k][:, tt_head:M_ROWS],
                        in_=tT_d.ap()[128 * k:128 * (k + 1), tt_head:M_ROWS])
            for n in range(4, N_NT):
                for k in range(N_KC):
                    vt_chunk(k, n, n + 1, _dma_q[k % 2])

        # S|N accumulator, side-major: col = side*(256*N_MT) + m*256 + n*32 + b
        sn_all = persist.tile([128, 2 * N_MT * 256], f32, tag="sn_all")
        # final staging, (j,q)-partition layout: row = 32j+q, col = 32h+b'
        # (video index = 128h + 32j + b'); avoids any partition-crossing DMA
        # until the single strided output DMA
        vt2_full = persist.tile([128, 64], f32, tag="vt2_full")
        bias0 = bias_sb[:, 0:1]

        def issue_sel(den2, num2, item):
            """The 4 over-t selector matmuls for one queued pair-tile.
            32-wide stationary: band j of den2/num2 is its own region-scoped
            accumulation group, complete at (mp==N_PAIR-1, mi==1)."""
            j, mp, exe = item
            for mi in range(2):
                m = 2 * mp + mi
                selm = sel_sb[:, 32 * m:32 * (m + 1)]
                band = slice(32 * j, 32 * (j + 1))
                nc.tensor.matmul(
                    den2[band, :], selm, exe[:, 512 * mi:512 * (mi + 1)],
                    start=(mp == 0 and mi == 0), stop=(mp == N_PAIR - 1 and mi == 1),
                    skip_group_check=True, tile_position=(0, 32 * j),
                )
                nc.tensor.matmul(
                    num2[band, :], selm, exe[:, 1024 + 512 * mi:1536 + 512 * mi],
                    start=(mp == 0 and mi == 0), stop=(mp == N_PAIR - 1 and mi == 1),
                    skip_group_check=True, tile_position=(0, 32 * j),
                )

        for _rep in range(reps):
            # t2v second-level accumulators in the (j,q)/(h,b') layout:
            # row = 32j+q, col = 32h+b'
            den3 = dn3_pool.tile([128, 64], f32, tag="den3")
            num3 = dn3_pool.tile([128, 64], f32, tag="num3")

            # --- phase-2 (t2v second level) for one m-tile and one b-half;
            # interleaved into the main loop right where its inputs are
            # ready.  Split into an a-stage (DVE ratio + ACT exp) and a
            # b-stage (DVE weight product + PE matmuls) so tail emission can
            # interleave several chains without idling DVE on ACT hops.
            def t2v_level2_a(mp, h):
                """t2v second level for BOTH m-tiles of pair mp at [128,256]
                width (their sn_all columns complete simultaneously)."""
                hb = 128 * h
                sn4 = sn_all[:].rearrange("p (s m c) -> p s m c", s=2, m=N_MT)
                s1 = sn4[:, 0, 2 * mp:2 * mp + 2, hb:hb + 128]
                n1 = sn4[:, 1, 2 * mp:2 * mp + 2, hb:hb + 128]
                rs1 = t2v_pool.tile([128, 256], f32, tag="rs1")
                nc.vector.reciprocal(rs1[:], s1)
                t2v_t = t2v_pool.tile([128, 256], f32, tag="t2v_t")
                nc.vector.tensor_tensor(t2v_t[:], n1, rs1[:], op=MUL)
                w_t = w_pool.tile([128, 256], bf16, tag="w_t")
                nc.scalar.activation(w_t[:], t2v_t[:], EXP, bias=bias0,
                                     scale=TAU2)
                return t2v_t, w_t

            def t2v_level2_b(mp, h, t2v_t, w_t):
                xw_t = w_pool.tile([128, 256], bf16, tag="xw_t")
                with nc.allow_low_precision(reason="bf16 softmax weights"):
                    nc.vector.tensor_tensor(xw_t[:], t2v_t[:], w_t[:], op=MUL)
                hcol = slice(32 * h, 32 * h + 32)
                for mi in range(2):
                    m = 2 * mp + mi
                    selm = sel_sb[:, 32 * m:32 * (m + 1)]
                    for j in range(4):
                        band = slice(32 * j, 32 * (j + 1))
                        wcol = slice(128 * mi + 32 * j, 128 * mi + 32 * j + 32)
                        nc.tensor.matmul(
                            den3[band, hcol], selm, w_t[:, wcol],
                            start=(m == 0), stop=(m == N_MT - 1),
                            skip_group_check=True, tile_position=(0, 32 * j),
                        )
                        nc.tensor.matmul(
                            num3[band, hcol], selm, xw_t[:, wcol],
                            start=(m == 0), stop=(m == N_MT - 1),
                            skip_group_check=True, tile_position=(0, 32 * j),
                        )

            def t2v_level2(mp, h):
                t2v_t, w_t = t2v_level2_a(mp, h)
                t2v_level2_b(mp, h, t2v_t, w_t)

            # ---- main loop: halves (b 0:128 / 128:256) x M-pairs x N-tiles
            for h in range(2):
                den2 = dn2_pool.tile([128, 512], f32, tag="den2")
                num2 = dn2_pool.tile([128, 512], f32, tag="num2")
                sel_queue = []   # software pipeline: sel MMs run 2 tiles late
                for mp in range(N_PAIR):
                    for j in range(4):
                        n = 4 * h + j
                        # drain queued selector matmuls BEFORE this tile's
                        # einsum: their deps are >=1 tile old (>=4 normally),
                        # so they slot into the PE stream without waiting,
                        # and the last pair drains 2/tile so nearly nothing
                        # serializes after the final XE.
                        if _TUNE["pre_pop"]:
                            npop = 2 if mp == N_PAIR - 1 else (
                                1 if len(sel_queue) > 4 else 0)
                            for _ in range(npop):
                                if sel_queue:
                                    issue_sel(den2, num2, sel_queue.pop(0))
                        ps = ps_pool.tile([128, 1024], f32, tag="ps")
                        for mi in range(2):
                            m = 2 * mp + mi
                            for k in range(N_KC):
                                nc.tensor.matmul(
                                    ps[:, 512 * mi:512 * (mi + 1)],
                                    tt_tiles[k][:, 128 * m:128 * (m + 1)],
                                    vt_tiles[(k, n)],
                                    start=(k == 0),
                                    stop=(k == N_KC - 1),
                                )
                        # E|XE for the pair: [E(m0) E(m1) XE(m0) XE(m1)]
                        exe = e_pool.tile([128, 2048], bf16, tag="exe")
                        # the last pair of the half gates the v2t second
                        # level: split exp/mult per m-tile and multiply
                        # straight from PSUM (1x mode) so its selector
                        # matmuls issue ~1us earlier
                        tail_pair = (mp == N_PAIR - 1 and j == 3)
                        with nc.allow_low_precision(reason="bf16 E/XE"):
                            if tail_pair:
                                # both exps emitted adjacently: keeps the
                                # scheduler's coalesced ACT->PE wait from
                                # covering the selector matmuls in between
                                for mi in range(2):
                                    ms = slice(512 * mi, 512 * mi + 512)
                                    nc.scalar.activation(
                                        exe[:, ms], ps[:, ms], EXP,
                                        bias=bias0, scale=TAU)
                                for mi in range(2):
                                    ms = slice(512 * mi, 512 * mi + 512)
                                    xs = slice(1024 + 512 * mi,
                                               1536 + 512 * mi)
                                    nc.vector.scalar_tensor_tensor(
                                        out=exe[:, xs], in0=ps[:, ms],
                                        scalar=0.5, in1=exe[:, ms],
                                        op0=MUL, op1=MUL)
                            else:
                                nc.scalar.activation(
                                    exe[:, 0:1024], ps[:], EXP, bias=bias0,
                                    scale=TAU)
                                xbt = xb_pool.tile([128, 1024], bf16,
                                                   tag="xbt")
                                # the X-copy carries the global 0.5 fold
                                nc.scalar.activation(xbt[:], ps[:], CPY,
                                                     bias=0.0, scale=0.5)
                                nc.vector.tensor_tensor(
                                    exe[:, 1024:2048], xbt[:], exe[:, 0:1024],
                                    op=MUL)
                        # queue the 4 selector matmuls (over-t sums on PE),
                        # issued a few tiles later so their ACT/DVE deps are
                        # done by the time PE reaches them.  In the last pair
                        # the queue drains 2/tile so den2/num2 close right
                        # after the final XE instead of 4 items later.
                        sel_queue.append((j, mp, exe))
                        if len(sel_queue) > _TUNE.get("lag", 5) or tail_pair:
                            issue_sel(den2, num2, sel_queue.pop(0))
                        # sum over v: 4-step strided bf16 add-tree on DVE.
                        # tr: [0:1024]=(c4,b32,v8)  [1024:1536]=(c,b,4)
                        #     [1536:1792]=(c,b,2)
                        tr = tr_pool.tile([128, 1792], bf16, tag="tr")
                        ev16 = exe[:].rearrange("p (c b v) -> p c b v", c=4,
                                                v=16)
                        t8 = tr[:, 0:1024].rearrange("p (c b v) -> p c b v",
                                                     c=4, v=8)
                        t4 = tr[:, 1024:1536].rearrange("p (c b v) -> p c b v",
                                                        c=4, v=4)
                        t2 = tr[:, 1536:1792].rearrange("p (c b v) -> p c b v",
                                                        c=4, v=2)
                        with nc.allow_low_precision(reason="bf16 v-tree"):
                            nc.vector.tensor_tensor(
                                t8[:], ev16[:, :, :, 0:8], ev16[:, :, :, 8:16],
                                op=ADD)
                            nc.vector.tensor_tensor(
                                t4[:], t8[:, :, :, 0:4], t8[:, :, :, 4:8],
                                op=ADD)
                            nc.vector.tensor_tensor(
                                t2[:], t4[:, :, :, 0:2], t4[:, :, :, 2:4],
                                op=ADD)
                        # final add writes fp32 S|N: col = s*(256*N_MT)+m*256+n*32+b
                        snv = sn_all[:].rearrange(
                            "p (s m nb) -> p s m nb", s=2, m=N_MT)[
                            :, :, 2 * mp:2 * mp + 2, 32 * n:32 * (n + 1)]
                        nc.vector.tensor_tensor(
                            snv, t2[:, :, :, 0:1], t2[:, :, :, 1:2], op=ADD)
                    # this pair's half-columns are complete: run its t2v
                    # second level overlapped with the remaining pairs (the
                    # last pair's runs after the v2t chain below, which is
                    # the longer critical path at the end of the half)
                    if mp < N_PAIR - 1:
                        t2v_level2(mp, h)
                while sel_queue:
                    issue_sel(den2, num2, sel_queue.pop(0))

                # ---- second level of v2t for this half (softmax over v),
                # split into two column groups (b' 0:16 / 16:32) so the serial
                # chain pipelines and the last-half tail is ~2x shorter ----
                v_t = v_pool.tile([128, 512], f32, tag="v_t")
                exev = v_pool.tile([128, 1024], bf16, tag="exev")
                snv_t = v_pool.tile([128, 64], f32, tag="snv_t")
                ev16 = exev[:].rearrange("p (c b v) -> p c b v", c=2, v=16)
                sn8 = snv_t[:].rearrange("p (c b) -> p c b", c=2)
                # ACT (idle here) can evacuate den2/num2 from PSUM so the DVE
                # reciprocal+multiply read SBUF (no PSUM access penalty)
                if _TUNE.get("evac", False):
                    den2c = v_pool.tile([128, 512], f32, tag="den2c")
                    num2c = v_pool.tile([128, 512], f32, tag="num2c")
                    nc.scalar.activation(den2c[:], den2[:], CPY)
                    nc.scalar.activation(num2c[:], num2[:], CPY)
                else:
                    den2c, num2c = den2, num2

                rden2 = v_pool.tile([128, 512], f32, tag="rden2")

                def v2t_a(cg):
                    cs = slice(256 * cg, 256 * cg + 256)
                    nc.vector.reciprocal(rden2[:, cs], den2c[:, cs])
                    nc.vector.tensor_tensor(
                        v_t[:, cs], num2c[:, cs], rden2[:, cs], op=MUL)
                    nc.scalar.activation(
                        exev[:, cs], v_t[:, cs], EXP, bias=bias0, scale=TAU2)

                def v2t_b(cg):
                    cs = slice(256 * cg, 256 * cg + 256)
                    bs = slice(16 * cg, 16 * cg + 16)
                    with nc.allow_low_precision(reason="bf16 Ev/XEv"):
                        nc.vector.tensor_tensor(
                            exev[:, 512 + 256 * cg:768 + 256 * cg],
                            v_t[:, cs], exev[:, cs], op=MUL)
                    # single reduce (no fast modes either way) instead of an
                    # add-tree: 1 instruction of latency on the tail chain.
                    # Optionally on the (idle) Pool engine to relieve DVE,
                    # which is the throughput bound of this tail.
                    red_eng = nc.gpsimd if _TUNE.get("pool_red", False) \
                        else nc.vector
                    red_eng.reduce_sum(
                        out=sn8[:, :, bs],
                        in_=ev16[:, :, bs, :],
                        axis=AX)
                    # vt2 = Nv'/Sv (the final /2 arrived via the XE fold),
                    # written straight into the (j,q)/(h,b') staging tile
                    rsv_t = v_pool.tile([128, 32], f32, tag="rsv_t")
                    nc.vector.reciprocal(rsv_t[:, bs], snv_t[:, bs])
                    nc.vector.tensor_tensor(
                        vt2_full[:, 32 * h + 16 * cg:32 * h + 16 * cg + 16],
                        snv_t[:, 32:64][:, bs], rsv_t[:, bs], op=MUL)

                # interleave the v2t chains, the deferred last pair's t2v
                # level 2, and phase 3 so DVE always has ready work while
                # ACT exps round-trip
                mp_last = N_PAIR - 1
                v2t_a(0)
                ta = t2v_level2_a(mp_last, h)
                v2t_a(1)
                v2t_b(0)
                t2v_level2_b(mp_last, h, *ta)
                v2t_b(1)

                # ---- phase 3 for this half: t2v2 = 0.5*Num3/Den3, combine
                # with v2t, and DMA out[q, 128h+32j+b'].  den3/num3's h-region
                # groups stop at m==N_MT-1 just above, so the h=0 finale fully
                # overlaps the h=1 main loop. ----
                hcol = slice(32 * h, 32 * h + 32)
                rden3 = t2v_pool.tile([128, 32], f32, tag="rden3")
                nc.vector.reciprocal(rden3[:], den3[:, hcol])
                t2v2 = t2v_pool.tile([128, 32], f32, tag="t2v2")
                nc.vector.tensor_tensor(t2v2[:], num3[:, hcol], rden3[:],
                                        op=MUL)
                out_sb = t2v_pool.tile([128, 32], f32, tag="out_sb")
                nc.vector.tensor_tensor(out_sb[:], t2v2[:],
                                        vt2_full[:, hcol], op=ADD)
                out_ap = out_d.ap().rearrange(
                    "q (h j b) -> h j q b", h=2, j=4)[h:h + 1]
                nc.sync.dma_start(out=out_ap, in_=out_sb[:])

    nc.compile()
    return nc


def _get_program(n_pairs=3, reps=1, variant=0, **_ignored):
    key = (n_pairs, reps, variant)
    if key not in _PROGRAM_CACHE:
        _PROGRAM_CACHE[key] = _build_program(n_pairs, reps, variant)
    return _PROGRAM_CACHE[key]


def _l2norm(a):
    n = np.linalg.norm(a, axis=-1, keepdims=True)
    return a / np.maximum(n, EPS)


def _bf16(a):
    import ml_dtypes
    return a.astype(ml_dtypes.bfloat16)


def _assign_queries(lengths):
    """Greedy LPT: partition the A queries into N_CORES groups of A_LOC,
    minimizing the max total token count per core."""
    order = np.argsort(-lengths, kind="stable")
    core_sum = np.zeros(N_CORES, np.int64)
    core_cnt = np.zeros(N_CORES, np.int64)
    groups = [[] for _ in range(N_CORES)]
    for q in order:
        open_c = np.nonzero(core_cnt < A_LOC)[0]
        c = open_c[np.argmin(core_sum[open_c])]
        groups[c].append(int(q))
        core_sum[c] += lengths[q]
        core_cnt[c] += 1
    return groups, int(core_sum.max())


def prepare_inputs(text_feat, video_feat, text_mask):
    """Host-side shard/layout prep with valid-token compaction.
    Returns (in_maps for the 8 cores, query permutation, n_pairs)."""
    t = _l2norm(text_feat.astype(np.float32))          # [A, T, D]
    v = _l2norm(video_feat.astype(np.float32))         # [B, V, D]
    mask = text_mask.astype(np.float32)

    # bake mask values into the text rows: logits = (mask*t) . v exactly
    tm = t * mask[:, :, None]

    lengths = (mask != 0.0).sum(axis=1).astype(np.int64)   # valid tokens per query
    groups, max_rows = _assign_queries(lengths)
    n_pairs = max(1, -(-max_rows // 256))                   # ceil to 256-row pairs
    m_rows = 256 * n_pairs
    n_mt = 2 * n_pairs

    # video: [B, V, D] -> [D, B*V], shared by all cores
    vT = _bf16(np.ascontiguousarray(v.reshape(B * V_FRM, D).T))

    in_maps = []
    perm = np.empty(A, np.int64)
    for c in range(N_CORES):
        qs = groups[c]
        perm[c * A_LOC:(c + 1) * A_LOC] = qs
        rows = np.zeros((m_rows, D), np.float32)
        sel = np.zeros((128, n_mt * 32), np.float32)
        r = 0
        for jloc, q in enumerate(qs):
            idx = np.nonzero(mask[q] != 0.0)[0]
            k = len(idx)
            rows[r:r + k] = tm[q, idx]
            rr = np.arange(r, r + k)
            sel[rr % 128, (rr // 128) * 32 + jloc] = 1.0
            r += k
        tT = _bf16(np.ascontiguousarray(rows.T))            # [D, m_rows]
        bias = np.full((128, 1), SHIFT, np.float32)
        in_maps.append({"tT": tT, "vT": vT, "sel": _bf16(sel), "bias": bias})
    return in_maps, perm, n_pairs


def run(in_maps, n_pairs=3, trace=False, reps=1, variant=0, **kwargs):
    import concourse.mybir as mybir
    from concourse import bass_utils

    nc = _get_program(n_pairs=n_pairs, reps=reps, variant=variant)
    # pad inputs to the program's declared shapes (bias width varies by build)
    shapes = {}
    for alloc in nc.m.functions[0].allocations:
        if isinstance(alloc, mybir.MemoryLocationSet) and alloc.kind == "ExternalInput":
            shapes[alloc.memorylocations[0].name] = tuple(alloc.tensor_shape)
    fixed = []
    for m in in_maps:
        mm = {}
        for k, v in m.items():
            shp = shapes.get(k, tuple(v.shape))
            if tuple(v.shape) != shp:
                out = np.full(shp, SHIFT if k == "bias" else 0.0, v.dtype)
                sl = tuple(slice(0, min(s, t)) for s, t in zip(v.shape, shp))
                out[sl] = v[sl]
                mm[k] = out
            else:
                mm[k] = v
        fixed.append(mm)
    return bass_utils.run_bass_kernel_spmd(
        nc, fixed, core_ids=list(range(N_CORES)), trace=trace, **kwargs
    )


def kernel(text_feat, video_feat, text_mask):
    in_maps, perm, n_pairs = prepare_inputs(
        np.asarray(text_feat), np.asarray(video_feat), np.asarray(text_mask)
    )
    res = run(in_maps, n_pairs=n_pairs)
    packed = np.concatenate([res.results[c]["out"] for c in range(N_CORES)], axis=0)
    out = np.empty_like(packed)
    out[perm] = packed                      # un-permute queries to original order
    return out.astype(np.float32)
